# revision 55
# baseline (speedup 1.0000x reference)
"""Multi-head attention TRN2 kernel (8 NeuronCores, SPMD).

Problem: B=2, N=2048, D=1024, H=16 heads of dim 64, fp32, per-(b,h)
key-length masking (valid_len, length 32).

Sharding: batch*heads across 8 cores - core c handles batch b=c//4 and 4
heads ("slots", rank-aligned by valid_len so the SPMD trip counts stay
balanced).  Per core:

  phase P: K projection (w chunks contiguous in DRAM so the first matmul
    gates on one 64KB DMA), then Q; first xq chunks prefetched behind the
    xk stream.  Drains split over ScalarE+DVE, attention-chunk-0 first;
    Q's final c-iteration finishes the chunk-0 accumulators first and
    drains them inline, so the first attention S matmul doesn't sit
    behind the other 12 projection matmuls in the PE FIFO (~2us).
  phase A (attention, bf16 operands, f32 PSUM):
    The V projection is STREAMED INTO CHUNK 0 as 32 bank-aligned
    single c-steps, ONE per item over the first 75% - pairing them into
    8-matmul bursts starves the exp feed ~8.7us, and two steps on one
    item measurably costs too (V mms sit ahead of S in the PE FIFO);
    PV-drain bursts are skipped entirely on iterations that carry an
    outproj (same FIFO-starvation mechanism, ~2.5us per boundary).  PSUM: V time-shares the 4 "acc2" banks; the
    attention accumulators allocate lazily after V's last pass, so
    4 sT + 4 V/acc2 = 8 exactly.
    V1 blocks are [V_j (64 cols) | ones (64 cols)] so the PV matmul emits
    the softmax denominator PRE-BROADCAST in PSUM rows 64:127.
    Per 512-query chunk, (slot, key-tile) items go round-robin in
    slot-PAIRS: the two 64-row S matmuls land on disjoint PE row groups
    (tile_position from base_partition) and overlap when slack allows.
    exp(S/8 + bias) on ScalarE - the valid_len mask is a per-partition
    bias column (0/-30000); ScalarE is the pacing engine (~578ns/tile).
    PVs are emitted with hysteresis (accumulate >=14 pending, then drain
    4 ahead of each S pair): the resulting dense back-to-back matmul runs
    keep the PE_HAM activity window busy so the clock gate stays at 8/8 -
    without this the PE sticks at 1.2GHz for 10-80us stretches and the
    kernel is bistable between ~187us and ~220us.  The PV queue is GLOBAL
    across chunks, so chunk q+1's S/exp stream starts while chunk q's
    backlog drains and ScalarE never pauses at chunk boundaries.
    normalize per slot right after its last PV: PSUM->SBUF copy,
    reciprocal_approx_fast (DVE custom op; PSUM input mis-executes, so
    feed it SBUF), one tensor_mul.  When a chunk's last
    norm fires, its output projection becomes dependency-ready and is
    spread one 128-query tile per item iteration (a 16-matmul blob here
    starves ScalarE ~6us per boundary).  Tail: the final pair's den
    copies and all closing stage copies run on ScalarE (idle after the
    last exp), and 12 dummy matmuls overlap the final DVE norm chains so
    the closing output projection runs at the warm 8/8 clock (it
    otherwise re-throttles to ~427ns/mm).
Host sums the 4 per-core partials of each batch element (the unshard for
the row-sharded Wo) and gathers.

Known-bad variants (measured): nt=2 exp batching (2-bank sT slots,
bufs=2), confirmed twice - on the per-chunk AND the global-pend
structure (+30us at nominal clock): the ~76ns/tile exp-overhead saving
is swamped by the coarser 2-slot pipeline; Q chunks 1-3 as in-attention
bursts (sT-slot pinch stalls ScalarE ~3us each, net loss); PSUM matmul
outputs that are not bank-aligned silently corrupt (half-bank V
accumulators); start-of-kernel PE warm-up dummies (they queue behind
const-ap memsets + the ACT table-load DMA, start ~8us, can't complete a
contiguous 3.4us busy window before DMA-ready, and push the first real
matmul 3us later - net loss, the ~1.7us cold-start is floor).
"""

import sys
import numpy as np
from collections import deque
from contextlib import ExitStack

sys.path.insert(0, "/opt/trn_rl_repo")

import concourse.bass as bass  # noqa: E402
from concourse import bacc, mybir  # noqa: E402
import concourse.tile as tile  # noqa: E402
from concourse.bass_utils import run_bass_kernel_spmd  # noqa: E402

F32 = mybir.dt.float32
BF16 = mybir.dt.bfloat16
AF = mybir.ActivationFunctionType
NPBF16 = mybir.dt.np(BF16)

B, N, D, H = 2, 2048, 1024, 16
DH = 64
HPC = 4          # heads (slots) per core
NCORES = 8
QC = 512         # q chunk (matmul free dim)
NKT = N // 128   # 16 k tiles
NDC = D // 128   # 8 contraction chunks
MASK_BIAS = -30000.0
LOOKAHEAD = 3    # PV stagger (in items) behind S/exp emission

LAST_RESULTS = None  # BassKernelResults of the most recent run (for tooling)


def _build_program(trips):
    """trips: 4 ints (k-tile count per slot)."""
    nc = bacc.Bacc("TRN2", target_bir_lowering=False, debug=False,
                   num_devices=NCORES)

    xTq = nc.dram_tensor("xTq", [D, N], BF16, kind="ExternalInput")
    xTk = nc.dram_tensor("xTk", [D, N], BF16, kind="ExternalInput")
    xTv = nc.dram_tensor("xTv", [D, N], BF16, kind="ExternalInput")
    wq = nc.dram_tensor("wq", [NDC * 128, 256], BF16, kind="ExternalInput")
    wk = nc.dram_tensor("wk", [NDC * 128, 256], BF16, kind="ExternalInput")
    wv = nc.dram_tensor("wv", [NDC * 128, 256], BF16, kind="ExternalInput")
    wo = nc.dram_tensor("wo", [256, D], BF16, kind="ExternalInput")
    vmask = nc.dram_tensor("vmask", [128, HPC * NKT], F32, kind="ExternalInput")
    out = nc.dram_tensor("out", [N, D], BF16, kind="ExternalOutput")

    # flat item list per chunk: round-robin tiles across slots so adjacent
    # items hit different slots (independent chains)
    items = []
    for r in range(max(trips)):
        for j in range(HPC):
            if r < trips[j]:
                items.append((j, r))

    with tile.TileContext(nc) as tc:
        with ExitStack() as ctx:
            wpool = ctx.enter_context(tc.tile_pool(name="wpool", bufs=1))
            xpool = ctx.enter_context(tc.tile_pool(name="xpool", bufs=6))
            qkpool = ctx.enter_context(tc.tile_pool(name="qkpool", bufs=1))
            v1pool = ctx.enter_context(tc.tile_pool(name="v1pool", bufs=1))
            ptpool = ctx.enter_context(tc.tile_pool(name="ptpool", bufs=40))
            nrmpool = ctx.enter_context(tc.tile_pool(name="nrmpool", bufs=4))
            pbpool = ctx.enter_context(tc.tile_pool(name="pbpool", bufs=1))
            opool = ctx.enter_context(tc.tile_pool(name="opool", bufs=8))

            t_wk = wpool.tile([128, NDC * 256], BF16, tag="wk")
            t_wq = wpool.tile([128, NDC * 256], BF16, tag="wq")
            t_wv = wpool.tile([128, NDC * 256], BF16, tag="wv")
            t_wo = [wpool.tile([128, D], BF16, tag=f"wo{p}", name=f"t_wo{p}")
                    for p in range(2)]
            t_vm = wpool.tile([128, HPC * NKT], F32, tag="vm")

            # K^T/Q^T: [128 dims (2 slots), N] per slot-pair
            t_kT = [qkpool.tile([128, N], BF16, tag=f"kT{p}", name=f"t_kT{p}")
                    for p in range(2)]
            t_qT = [qkpool.tile([128, N], BF16, tag=f"qT{p}", name=f"t_qT{p}")
                    for p in range(2)]
            # V1: per key-tile t, 4 blocks of [V_j (64 cols) | ones (64 cols)]
            t_v1 = v1pool.tile([128, NKT * HPC * 128], BF16, tag="v1")
            # normalized heads^T per slot pair: [128 dims, N]
            t_pb = [pbpool.tile([128, N], BF16, tag=f"pb{p}", name=f"t_pb{p}")
                    for p in range(2)]
            # scratch for the ACT exp-table preload
            t_pre = wpool.tile([1, 1], F32, tag="pre")

            # ---- phase P: projections (K, Q, V) ----
            with tc.tile_pool(name="pp", bufs=8, space="PSUM") as pp:
                xq_pre = []
                for si, (xin, wsb, dsts) in enumerate(
                        ((xTk, t_wk, t_kT), (xTq, t_wq, t_qT))):
                    accs = [pp.tile([128, QC], F32, tag="acc", name=f"acc_{i}")
                            for i in range(8)]
                    for c in range(NDC):
                        if si == 0 and c >= 4:
                            # prefetch the first xq chunks behind the xk
                            # stream so Q matmuls start right after K's
                            xp = xpool.tile([128, N], BF16, tag="xqp",
                                            bufs=4)
                            nc.sync.dma_start(
                                xp[:], xTq[(c - 4) * 128:(c - 3) * 128, :])
                            xq_pre.append(xp)
                        if si == 1 and c < 4:
                            xt = xq_pre[c]
                        else:
                            xt = xpool.tile([128, N], BF16, tag="xt")
                        if si == 0 and c == 0:
                            # split the gating first chunk across 4 DMA
                            # queues so the first matmul fires sooner
                            for sp in range(4):
                                cs = slice(sp * 512, (sp + 1) * 512)
                                nc.sync.dma_start(
                                    xt[:, cs], xin[0:128, cs])
                        elif not (si == 1 and c < 4):
                            nc.sync.dma_start(
                                xt[:], xin[c * 128:(c + 1) * 128, :])
                        if si == 0:
                            # wk sliced per chunk: the first matmul only
                            # gates on 64KB of weights + one x chunk
                            nc.sync.dma_start(
                                t_wk[:, c * 256:(c + 1) * 256],
                                wk[c * 128:(c + 1) * 128, :])
                        if si == 0 and c == 0:
                            # ACT exp-table preload + V1 ones memset, behind
                            # the first input DMAs in trigger order so they
                            # don't delay the x stream
                            nc.scalar.activation(t_pre[:], t_pre[:], AF.Exp)
                            ones_ap = t_v1[:].rearrange(
                                "p (b c) -> p b c", c=128)[:, :, 64:128]
                            nc.vector.memset(ones_ap, 1.0)
                        qq_order = range(4)
                        if si == 1 and c == NDC - 1:
                            # finish the chunk-0 (qq=0) accumulators
                            # first and drain them inline, so the first
                            # attention S matmul doesn't sit behind the
                            # other 12 projection matmuls in the PE FIFO
                            for m in range(2):
                                nc.tensor.matmul(
                                    accs[m * 4][:],
                                    wsb[:, c * 256 + m * 128:
                                        c * 256 + (m + 1) * 128],
                                    xt[:, 0:QC],
                                    start=False, stop=True)
                            with nc.allow_low_precision(reason="f32r 4B"):
                                nc.scalar.activation(
                                    dsts[0][:, 0:QC], accs[0][:], AF.Copy)
                                nc.vector.tensor_copy(
                                    dsts[1][:, 0:QC], accs[4][:])
                            qq_order = range(1, 4)
                        for m in range(2):
                            for qq in qq_order:
                                nc.tensor.matmul(
                                    accs[m * 4 + qq][:],
                                    wsb[:, c * 256 + m * 128:
                                        c * 256 + (m + 1) * 128],
                                    xt[:, qq * QC:(qq + 1) * QC],
                                    start=(c == 0), stop=(c == NDC - 1))
                    wnext, tnext = (wq, t_wq) if si == 0 else (wv, t_wv)
                    for c in range(NDC):
                        nc.sync.dma_start(
                            tnext[:, c * 256:(c + 1) * 256],
                            wnext[c * 128:(c + 1) * 128, :])
                    # drain PSUM->SBUF casts on BOTH ScalarE and DVE, q-chunk
                    # 0 first: the first attention S matmuls gate only on the
                    # chunk-0 casts, so phase A starts ~2.5us earlier
                    order = (0, 4, 1, 5, 2, 6, 3, 7) if si == 0 else (
                        1, 5, 2, 6, 3, 7)
                    for n, i in enumerate(order):
                        dst = dsts[i // 4][:, (i % 4) * QC:(i % 4 + 1) * QC]
                        with nc.allow_low_precision(reason="f32r 4B"):
                            if n % 2 == 0:
                                nc.scalar.activation(dst, accs[i][:], AF.Copy)
                            else:
                                nc.vector.tensor_copy(dst, accs[i][:])
                nc.sync.dma_start(t_wo[0][:], wo[0:128, :])
                nc.sync.dma_start(t_wo[1][:], wo[128:256, :])
                nc.sync.dma_start(t_vm[:], vmask[:])

            # ---- phase A: attention with V projection streamed into chunk
            # 0 (fills PE bubbles while ScalarE chews chunk-0 exps) and
            # fused output projection ----
            with tc.tile_pool(name="ap", bufs=1, space="PSUM") as ap:
                # V projection, split into 16 c-steps interleaved with the
                # chunk-0 S/exp stream.  PSUM: the two 4-bank pass groups
                # come from the same rotation ("acc2" tag) that the
                # attention accumulators use later - acc2 tiles allocate
                # only after V is done, so 4 sT + 4 V banks = 8 exactly.
                vaccs = [None]

                def v_step(sp, c):
                    # sub-pass sp covers 4 key-tiles (bank-aligned [128,512]
                    # slots, V data in cols 0:256), streamed over 8 c-steps
                    g, h = sp // 2, sp % 2
                    if c == 0:
                        vaccs[0] = [ap.tile([128, 512], F32, tag="acc2",
                                            bufs=4, name=f"vacc{sp}_{i}")
                                    for i in range(4)]
                    xt = xpool.tile([128, 512], BF16, tag="xtv")
                    nc.sync.dma_start(
                        xt[:], xTv[c * 128:(c + 1) * 128,
                                   g * 1024 + h * 512:
                                   g * 1024 + (h + 1) * 512])
                    for k in range(4):
                        nc.tensor.matmul(
                            vaccs[0][k][:, 0:256],
                            xt[:, k * 128:(k + 1) * 128],
                            t_wv[:, c * 256:(c + 1) * 256],
                            start=(c == 0), stop=(c == NDC - 1))
                    if c == NDC - 1:
                        for k in range(4):
                            t = g * 8 + h * 4 + k
                            # [128, 4, 64] strided copy: slot j -> V1 block
                            src = vaccs[0][k][:, 0:256].rearrange(
                                "p (j c) -> p j c", c=64)
                            dst = t_v1[:, t * 512:(t + 1) * 512].rearrange(
                                "p (j c) -> p j c", c=128)[:, :, 0:64]
                            with nc.allow_low_precision(reason="f32r 4B"):
                                nc.vector.tensor_copy(dst, src)
                def emit_outproj_qt(qt, scalar_copy=False):
                    ts = slice(qt * 128, (qt + 1) * 128)
                    stage = opool.tile([128, D], BF16, tag="ostage")
                    o_ps = [ap.tile([128, 512], F32, tag="sT", bufs=4,
                                    name=f"o_ps{ch}") for ch in range(2)]
                    for p2 in (1, 0):
                        for ch in range(2):
                            nc.tensor.matmul(
                                o_ps[ch][:], t_pb[p2][:, ts],
                                t_wo[p2][:, ch * 512:(ch + 1) * 512],
                                start=(p2 == 1), stop=(p2 == 0))
                    for ch in range(2):
                        dst = stage[:, ch * 512:(ch + 1) * 512]
                        with nc.allow_low_precision(reason="bf16 out"):
                            if scalar_copy:
                                # tail only: ScalarE is idle after the
                                # last exp; parallels the DVE norm chains
                                nc.scalar.activation(
                                    dst, o_ps[ch][:], AF.Copy)
                            else:
                                nc.vector.tensor_copy(dst, o_ps[ch][:])
                    nc.sync.dma_start(out[ts, :], stage[:])

                nitems = len(items)
                # global PV queue across chunks: chunk q+1's S/exp stream
                # starts while chunk q's PV backlog drains, so ScalarE
                # never pauses at chunk boundaries
                pend = deque()
                drain_mode = [False]
                vleft = [32]

                def drain_policy(q, k):
                    # hysteresis: accumulate PVs, then emit them in dense
                    # runs (ahead of the next S pair) - the back-to-back
                    # matmul bursts keep the PE activity window busy so
                    # the HAM clock gate stays at 8/8; in the last chunk's
                    # second half, drain continuously to flatten the tail
                    if len(pend) >= 14:
                        drain_mode[0] = True
                    if len(pend) <= LOOKAHEAD:
                        drain_mode[0] = False
                    r = 4 if drain_mode[0] else 0
                    if q and k < 12:
                        # boundary window: the previous chunk's backlog +
                        # outprojs already crowd the PE FIFO - cap drains
                        # so the new chunk's exp feed doesn't slip
                        # (~2.5us gap at each boundary otherwise)
                        r = min(r, 2)
                    return r

                op_queue = []

                def emit_pv(ctx, j, t, pt):
                    if not ctx["accs2"]:
                        ctx["accs2"].extend(
                            ap.tile([128, QC], F32, tag="acc2", bufs=4,
                                    name=f"acc_{jj}")
                            for jj in range(HPC))
                    accs2, seen, qs = ctx["accs2"], ctx["seen"], ctx["qs"]
                    base = (t * HPC + j) * 128
                    seen[j] += 1
                    nc.tensor.matmul(
                        accs2[j][:], t_v1[:, base:base + 128], pt[:],
                        start=(seen[j] == 1), stop=(seen[j] == trips[j]))
                    if seen[j] == trips[j]:
                        # normalize right after the slot's last PV:
                        # denominator is pre-broadcast in rows 64:127
                        p, half = j // 2, j % 2
                        rows = slice(half * 64, (half + 1) * 64)
                        den = nrmpool.tile([64, QC], F32, tag="den")
                        if ctx["q"] == N // QC - 1 and j < 2:
                            # final pair: ScalarE is idle after the last
                            # exp - its copy shortens the DVE-serial
                            # norm chain on the critical tail path
                            nc.scalar.activation(
                                den[:], accs2[j][64:128, :], AF.Copy)
                        else:
                            nc.vector.tensor_copy(
                                den[:], accs2[j][64:128, :])
                        rcp = nrmpool.tile([64, QC], F32, tag="rcp")
                        nc.vector.reciprocal_approx_fast(rcp[:], den[:])
                        with nc.allow_low_precision(reason="f32r 4B"):
                            nc.vector.tensor_mul(
                                t_pb[p][rows, qs], accs2[j][0:64, :],
                                rcp[:])
                        if all(ctx["seen"][jj] == trips[jj]
                               for jj in range(HPC)):
                            # all four slots normalized: the chunk's
                            # output projection is dependency-ready;
                            # spread it one query-tile per item iteration
                            op_queue.extend(
                                range(ctx["q"] * 4, (ctx["q"] + 1) * 4))

                for q in range(N // QC):
                    qs = slice(q * QC, (q + 1) * QC)
                    ctx = {"q": q, "qs": qs, "accs2": [],
                           "seen": [0] * HPC}

                    # chunk 0 carries the 32 V-projection c-steps spread
                    # over its first ~3/4 items (PVs defer until V's PSUM
                    # banks are free)
                    vsched = {}
                    if q == 0:
                        # single c-steps (4 matmuls): paired 8-matmul
                        # V bursts sit ahead of the S matmuls in the PE
                        # FIFO and starve the exp feed (~8.7us of chunk-0
                        # exp gaps measured with pairs)
                        span = max(32, int(nitems * 0.75))
                        for s in range(32):
                            vsched.setdefault(
                                min(1 + s * span // 32, nitems - 1), []
                            ).append(s)

                    k = 0
                    while k < nitems:
                        # S pair + exps FIRST: a drain burst ahead of the
                        # S's in the PE FIFO delays the exp feed by the
                        # burst length (measured as ~12 gaps of 1-3us at
                        # burst-onset/outproj points)
                        batch = items[k:k + 2]
                        sts = []
                        for (j, t) in batch:
                            p, half = j // 2, j % 2
                            rows = slice(half * 64, (half + 1) * 64)
                            sT = ap.tile([128, QC], F32, tag="sT", bufs=4)
                            nc.tensor.matmul(
                                sT[:], t_kT[p][rows, t * 128:(t + 1) * 128],
                                t_qT[p][rows, qs], start=True, stop=True)
                            sts.append(sT)
                        for (j, t), sT in zip(batch, sts):
                            pT = ptpool.tile([128, QC], BF16, tag="pT")
                            nc.scalar.activation(
                                pT[:], sT[:], AF.Exp, scale=0.125,
                                bias=t_vm[:, j * NKT + t: j * NKT + t + 1])
                            pend.append((ctx, j, t, pT))
                        for i in (k, k + 1):
                            for s in vsched.get(i, ()):
                                v_step(s // 8, s % 8)
                                vleft[0] -= 1
                        if vleft[0] == 0:
                            budget = drain_policy(q, k)
                            if op_queue:
                                # an outproj (4 mms) stacks on this
                                # iteration - skip the PV burst so the
                                # S feed isn't starved
                                budget = 0
                            while len(pend) > LOOKAHEAD and budget:
                                emit_pv(*pend.popleft())
                                budget -= 1
                            if op_queue:
                                emit_outproj_qt(op_queue.pop(0))
                        k += 2
                while pend:
                    emit_pv(*pend.popleft())
                # warm-keeper: dense dummy matmuls (dead sT slot, no data
                # deps) execute in parallel with the final norm chains on
                # DVE, so the closing output projection runs at the warm
                # 8/8 clock instead of re-throttled 4/8 (~427ns/mm)
                warm = ap.tile([128, QC], F32, tag="sT", bufs=4)
                for _ in range(12):
                    nc.tensor.matmul(
                        warm[:], t_wo[0][:, 0:128], t_qT[0][:, 0:QC],
                        start=True, stop=True)
                for qt in op_queue:
                    emit_outproj_qt(qt, scalar_copy=True)

    nc.finalize()
    return nc


def _make_plans(trips, vls_by_slot):
    """Greedy pair batching: (t, t+1) share one exp iff every core's vl is
    outside the open interval (128*t, 128*(t+2)) - then one bias column
    describes both tiles on every core."""
    plans = []
    for j in range(HPC):
        plan, t = [], 0
        while t < trips[j]:
            if t + 1 < trips[j] and all(
                    v <= 128 * t or v >= 128 * (t + 2)
                    for v in vls_by_slot[j]):
                plan.append((t, 2))
                t += 2
            else:
                plan.append((t, 1))
                t += 1
        plans.append(plan)
    return plans


def kernel(queries, keys, values, valid_len, Wq, Wk, Wv, Wo):
    global LAST_RESULTS
    queries = np.asarray(queries, dtype=np.float32)
    keys = np.asarray(keys, dtype=np.float32)
    values = np.asarray(values, dtype=np.float32)
    Wq = np.asarray(Wq, dtype=np.float32)
    Wk = np.asarray(Wk, dtype=np.float32)
    Wv = np.asarray(Wv, dtype=np.float32)
    Wo = np.asarray(Wo, dtype=np.float32)
    vl = np.asarray(valid_len).astype(np.int64).reshape(B * H)

    # rank-aligned slot assignment: per batch, heads sorted by vl desc;
    # slot j of the 4 cores of that batch takes ranks 4j..4j+3
    order = {}
    for b in range(B):
        idx = (np.argsort(-vl[b * H:(b + 1) * H], kind="stable") + b * H)
        for cg in range(4):
            order[b * 4 + cg] = [int(idx[4 * j + cg]) for j in range(HPC)]
    trips, vls_by_slot = [], []
    for j in range(HPC):
        vs = [int(vl[order[c][j]]) for c in range(NCORES)]
        vls_by_slot.append(vs)
        m = max(-(-v // 128) for v in vs)
        trips.append(max(1, min(NKT, m)))
    nc = _build_program(tuple(trips))

    in_maps = []
    for c in range(NCORES):
        b = c // 4
        heads = order[c]
        cols = np.concatenate(
            [np.arange((h - b * H) * DH, (h - b * H + 1) * DH) for h in heads])

        def wlayout(w):
            return np.ascontiguousarray(
                w[:, cols].reshape(NDC * 128, 256).astype(NPBF16))

        vm = np.zeros((128, HPC * NKT), np.float32)
        for j, h in enumerate(heads):
            bias = np.where(np.arange(N) < vl[h], 0.0, MASK_BIAS)
            vm[:, j * NKT:(j + 1) * NKT] = bias.reshape(NKT, 128).T

        in_maps.append({
            "xTq": np.ascontiguousarray(queries[b].T.astype(NPBF16)),
            "xTk": np.ascontiguousarray(keys[b].T.astype(NPBF16)),
            "xTv": np.ascontiguousarray(values[b].T.astype(NPBF16)),
            "wq": wlayout(Wq),
            "wk": wlayout(Wk),
            "wv": wlayout(Wv),
            "wo": np.ascontiguousarray(Wo[cols, :]).astype(NPBF16),
            "vmask": vm,
        })

    LAST_RESULTS = run_bass_kernel_spmd(nc, in_maps, list(range(NCORES)))
    res = LAST_RESULTS.results

    out = np.zeros((B, N, D), np.float64)
    for c in range(NCORES):
        out[c // 4] += res[c]["out"].astype(np.float64)
    return out.astype(np.float32)


# revision 56
# speedup vs baseline: 1.0110x; 1.0110x over previous
"""Multi-head attention TRN2 kernel (8 NeuronCores, SPMD).

Problem: B=2, N=2048, D=1024, H=16 heads of dim 64, fp32, per-(b,h)
key-length masking (valid_len, length 32).

Sharding: batch*heads across 8 cores - core c handles batch b=c//4 and 4
heads ("slots", rank-aligned by valid_len so the SPMD trip counts stay
balanced).  Per core:

  phase P: K projection (w chunks contiguous in DRAM so the first matmul
    gates on one 64KB DMA), then Q; first xq chunks prefetched behind the
    xk stream.  Drains split over ScalarE+DVE, attention-chunk-0 first;
    Q's final c-iteration finishes the chunk-0 accumulators first and
    drains them inline, so the first attention S matmul doesn't sit
    behind the other 12 projection matmuls in the PE FIFO (~2us).
  phase A (attention, bf16 operands, f32 PSUM):
    The V projection is STREAMED INTO CHUNK 0 as 32 bank-aligned
    single c-steps, ONE per item over the first 75% - pairing them into
    8-matmul bursts starves the exp feed ~8.7us, and two steps on one
    item measurably costs too (V mms sit ahead of S in the PE FIFO);
    PV-drain bursts are skipped entirely on iterations that carry an
    outproj (same FIFO-starvation mechanism, ~2.5us per boundary).  PSUM: V time-shares the 4 "acc2" banks; the
    attention accumulators allocate lazily after V's last pass, so
    4 sT + 4 V/acc2 = 8 exactly.
    V1 blocks are [V_j (64 cols) | ones (64 cols)] so the PV matmul emits
    the softmax denominator PRE-BROADCAST in PSUM rows 64:127.
    Per 512-query chunk, (slot, key-tile) items go round-robin in
    slot-PAIRS: the two 64-row S matmuls land on disjoint PE row groups
    (tile_position from base_partition) and overlap when slack allows.
    exp(S/8 + bias) on ScalarE - the valid_len mask is a per-partition
    bias column (0/-30000); ScalarE is the pacing engine (~578ns/tile).
    PVs are emitted with hysteresis (accumulate >=14 pending, then drain
    4 ahead of each S pair): the resulting dense back-to-back matmul runs
    keep the PE_HAM activity window busy so the clock gate stays at 8/8 -
    without this the PE sticks at 1.2GHz for 10-80us stretches and the
    kernel is bistable between ~187us and ~220us.  The PV queue is GLOBAL
    across chunks, so chunk q+1's S/exp stream starts while chunk q's
    backlog drains and ScalarE never pauses at chunk boundaries.
    normalize per slot right after its last PV: PSUM->SBUF copy,
    reciprocal_approx_fast (DVE custom op; PSUM input mis-executes, so
    feed it SBUF), one tensor_mul.  When a chunk's last
    norm fires, its output projection becomes dependency-ready and is
    spread one 128-query tile per item iteration (a 16-matmul blob here
    starves ScalarE ~6us per boundary).  Tail: the final pair's den
    copies and all closing stage copies run on ScalarE (idle after the
    last exp), and 12 dummy matmuls overlap the final DVE norm chains so
    the closing output projection runs at the warm 8/8 clock (it
    otherwise re-throttles to ~427ns/mm).
Host sums the 4 per-core partials of each batch element (the unshard for
the row-sharded Wo) and gathers.

Known-bad variants (measured): nt=2 exp batching (2-bank sT slots,
bufs=2), confirmed twice - on the per-chunk AND the global-pend
structure (+30us at nominal clock): the ~76ns/tile exp-overhead saving
is swamped by the coarser 2-slot pipeline; Q chunks 1-3 as in-attention
bursts (sT-slot pinch stalls ScalarE ~3us each, net loss); PSUM matmul
outputs that are not bank-aligned silently corrupt (half-bank V
accumulators); start-of-kernel PE warm-up dummies (they queue behind
const-ap memsets + the ACT table-load DMA, start ~8us, can't complete a
contiguous 3.4us busy window before DMA-ready, and push the first real
matmul 3us later - net loss, the ~1.7us cold-start is floor); capping
PV drains at 2 for each chunk's first 6 iterations (targeting the three
~2.5us boundary gaps) - nominal-clock regression to ~199us: the slower
backlog drain thins the dense PV bursts that keep the clock gate warm.
"""

import sys
import numpy as np
from collections import deque
from contextlib import ExitStack

sys.path.insert(0, "/opt/trn_rl_repo")

import concourse.bass as bass  # noqa: E402
from concourse import bacc, mybir  # noqa: E402
import concourse.tile as tile  # noqa: E402
from concourse.bass_utils import run_bass_kernel_spmd  # noqa: E402

F32 = mybir.dt.float32
BF16 = mybir.dt.bfloat16
AF = mybir.ActivationFunctionType
NPBF16 = mybir.dt.np(BF16)

B, N, D, H = 2, 2048, 1024, 16
DH = 64
HPC = 4          # heads (slots) per core
NCORES = 8
QC = 512         # q chunk (matmul free dim)
NKT = N // 128   # 16 k tiles
NDC = D // 128   # 8 contraction chunks
MASK_BIAS = -30000.0
LOOKAHEAD = 3    # PV stagger (in items) behind S/exp emission

LAST_RESULTS = None  # BassKernelResults of the most recent run (for tooling)


def _build_program(trips):
    """trips: 4 ints (k-tile count per slot)."""
    nc = bacc.Bacc("TRN2", target_bir_lowering=False, debug=False,
                   num_devices=NCORES)

    xTq = nc.dram_tensor("xTq", [D, N], BF16, kind="ExternalInput")
    xTk = nc.dram_tensor("xTk", [D, N], BF16, kind="ExternalInput")
    xTv = nc.dram_tensor("xTv", [D, N], BF16, kind="ExternalInput")
    wq = nc.dram_tensor("wq", [NDC * 128, 256], BF16, kind="ExternalInput")
    wk = nc.dram_tensor("wk", [NDC * 128, 256], BF16, kind="ExternalInput")
    wv = nc.dram_tensor("wv", [NDC * 128, 256], BF16, kind="ExternalInput")
    wo = nc.dram_tensor("wo", [256, D], BF16, kind="ExternalInput")
    vmask = nc.dram_tensor("vmask", [128, HPC * NKT], F32, kind="ExternalInput")
    out = nc.dram_tensor("out", [N, D], BF16, kind="ExternalOutput")

    # flat item list per chunk: round-robin tiles across slots so adjacent
    # items hit different slots (independent chains)
    items = []
    for r in range(max(trips)):
        for j in range(HPC):
            if r < trips[j]:
                items.append((j, r))

    with tile.TileContext(nc) as tc:
        with ExitStack() as ctx:
            wpool = ctx.enter_context(tc.tile_pool(name="wpool", bufs=1))
            xpool = ctx.enter_context(tc.tile_pool(name="xpool", bufs=6))
            qkpool = ctx.enter_context(tc.tile_pool(name="qkpool", bufs=1))
            v1pool = ctx.enter_context(tc.tile_pool(name="v1pool", bufs=1))
            ptpool = ctx.enter_context(tc.tile_pool(name="ptpool", bufs=40))
            nrmpool = ctx.enter_context(tc.tile_pool(name="nrmpool", bufs=4))
            pbpool = ctx.enter_context(tc.tile_pool(name="pbpool", bufs=1))
            opool = ctx.enter_context(tc.tile_pool(name="opool", bufs=8))

            t_wk = wpool.tile([128, NDC * 256], BF16, tag="wk")
            t_wq = wpool.tile([128, NDC * 256], BF16, tag="wq")
            t_wv = wpool.tile([128, NDC * 256], BF16, tag="wv")
            t_wo = [wpool.tile([128, D], BF16, tag=f"wo{p}", name=f"t_wo{p}")
                    for p in range(2)]
            t_vm = wpool.tile([128, HPC * NKT], F32, tag="vm")

            # K^T/Q^T: [128 dims (2 slots), N] per slot-pair
            t_kT = [qkpool.tile([128, N], BF16, tag=f"kT{p}", name=f"t_kT{p}")
                    for p in range(2)]
            t_qT = [qkpool.tile([128, N], BF16, tag=f"qT{p}", name=f"t_qT{p}")
                    for p in range(2)]
            # V1: per key-tile t, 4 blocks of [V_j (64 cols) | ones (64 cols)]
            t_v1 = v1pool.tile([128, NKT * HPC * 128], BF16, tag="v1")
            # normalized heads^T per slot pair: [128 dims, N]
            t_pb = [pbpool.tile([128, N], BF16, tag=f"pb{p}", name=f"t_pb{p}")
                    for p in range(2)]
            # scratch for the ACT exp-table preload
            t_pre = wpool.tile([1, 1], F32, tag="pre")

            # ---- phase P: projections (K, Q, V) ----
            with tc.tile_pool(name="pp", bufs=8, space="PSUM") as pp:
                xq_pre = []
                for si, (xin, wsb, dsts) in enumerate(
                        ((xTk, t_wk, t_kT), (xTq, t_wq, t_qT))):
                    accs = [pp.tile([128, QC], F32, tag="acc", name=f"acc_{i}")
                            for i in range(8)]
                    for c in range(NDC):
                        if si == 0 and c >= 4:
                            # prefetch the first xq chunks behind the xk
                            # stream so Q matmuls start right after K's
                            xp = xpool.tile([128, N], BF16, tag="xqp",
                                            bufs=4)
                            nc.sync.dma_start(
                                xp[:], xTq[(c - 4) * 128:(c - 3) * 128, :])
                            xq_pre.append(xp)
                        if si == 1 and c < 4:
                            xt = xq_pre[c]
                        else:
                            xt = xpool.tile([128, N], BF16, tag="xt")
                        if si == 0 and c == 0:
                            # split the gating first chunk across 4 DMA
                            # queues so the first matmul fires sooner
                            for sp in range(4):
                                cs = slice(sp * 512, (sp + 1) * 512)
                                nc.sync.dma_start(
                                    xt[:, cs], xin[0:128, cs])
                        elif not (si == 1 and c < 4):
                            nc.sync.dma_start(
                                xt[:], xin[c * 128:(c + 1) * 128, :])
                        if si == 0:
                            # wk sliced per chunk: the first matmul only
                            # gates on 64KB of weights + one x chunk
                            nc.sync.dma_start(
                                t_wk[:, c * 256:(c + 1) * 256],
                                wk[c * 128:(c + 1) * 128, :])
                        if si == 0 and c == 0:
                            # ACT exp-table preload + V1 ones memset, behind
                            # the first input DMAs in trigger order so they
                            # don't delay the x stream
                            nc.scalar.activation(t_pre[:], t_pre[:], AF.Exp)
                            ones_ap = t_v1[:].rearrange(
                                "p (b c) -> p b c", c=128)[:, :, 64:128]
                            nc.vector.memset(ones_ap, 1.0)
                        qq_order = range(4)
                        if si == 1 and c == NDC - 1:
                            # finish the chunk-0 (qq=0) accumulators
                            # first and drain them inline, so the first
                            # attention S matmul doesn't sit behind the
                            # other 12 projection matmuls in the PE FIFO
                            for m in range(2):
                                nc.tensor.matmul(
                                    accs[m * 4][:],
                                    wsb[:, c * 256 + m * 128:
                                        c * 256 + (m + 1) * 128],
                                    xt[:, 0:QC],
                                    start=False, stop=True)
                            with nc.allow_low_precision(reason="f32r 4B"):
                                nc.scalar.activation(
                                    dsts[0][:, 0:QC], accs[0][:], AF.Copy)
                                nc.vector.tensor_copy(
                                    dsts[1][:, 0:QC], accs[4][:])
                            qq_order = range(1, 4)
                        for m in range(2):
                            for qq in qq_order:
                                nc.tensor.matmul(
                                    accs[m * 4 + qq][:],
                                    wsb[:, c * 256 + m * 128:
                                        c * 256 + (m + 1) * 128],
                                    xt[:, qq * QC:(qq + 1) * QC],
                                    start=(c == 0), stop=(c == NDC - 1))
                    wnext, tnext = (wq, t_wq) if si == 0 else (wv, t_wv)
                    for c in range(NDC):
                        nc.sync.dma_start(
                            tnext[:, c * 256:(c + 1) * 256],
                            wnext[c * 128:(c + 1) * 128, :])
                    # drain PSUM->SBUF casts on BOTH ScalarE and DVE, q-chunk
                    # 0 first: the first attention S matmuls gate only on the
                    # chunk-0 casts, so phase A starts ~2.5us earlier
                    order = (0, 4, 1, 5, 2, 6, 3, 7) if si == 0 else (
                        1, 5, 2, 6, 3, 7)
                    for n, i in enumerate(order):
                        dst = dsts[i // 4][:, (i % 4) * QC:(i % 4 + 1) * QC]
                        with nc.allow_low_precision(reason="f32r 4B"):
                            if n % 2 == 0:
                                nc.scalar.activation(dst, accs[i][:], AF.Copy)
                            else:
                                nc.vector.tensor_copy(dst, accs[i][:])
                nc.sync.dma_start(t_wo[0][:], wo[0:128, :])
                nc.sync.dma_start(t_wo[1][:], wo[128:256, :])
                nc.sync.dma_start(t_vm[:], vmask[:])

            # ---- phase A: attention with V projection streamed into chunk
            # 0 (fills PE bubbles while ScalarE chews chunk-0 exps) and
            # fused output projection ----
            with tc.tile_pool(name="ap", bufs=1, space="PSUM") as ap:
                # V projection, split into 16 c-steps interleaved with the
                # chunk-0 S/exp stream.  PSUM: the two 4-bank pass groups
                # come from the same rotation ("acc2" tag) that the
                # attention accumulators use later - acc2 tiles allocate
                # only after V is done, so 4 sT + 4 V banks = 8 exactly.
                vaccs = [None]

                def v_step(sp, c):
                    # sub-pass sp covers 4 key-tiles (bank-aligned [128,512]
                    # slots, V data in cols 0:256), streamed over 8 c-steps
                    g, h = sp // 2, sp % 2
                    if c == 0:
                        vaccs[0] = [ap.tile([128, 512], F32, tag="acc2",
                                            bufs=4, name=f"vacc{sp}_{i}")
                                    for i in range(4)]
                    xt = xpool.tile([128, 512], BF16, tag="xtv")
                    nc.sync.dma_start(
                        xt[:], xTv[c * 128:(c + 1) * 128,
                                   g * 1024 + h * 512:
                                   g * 1024 + (h + 1) * 512])
                    for k in range(4):
                        nc.tensor.matmul(
                            vaccs[0][k][:, 0:256],
                            xt[:, k * 128:(k + 1) * 128],
                            t_wv[:, c * 256:(c + 1) * 256],
                            start=(c == 0), stop=(c == NDC - 1))
                    if c == NDC - 1:
                        for k in range(4):
                            t = g * 8 + h * 4 + k
                            # [128, 4, 64] strided copy: slot j -> V1 block
                            src = vaccs[0][k][:, 0:256].rearrange(
                                "p (j c) -> p j c", c=64)
                            dst = t_v1[:, t * 512:(t + 1) * 512].rearrange(
                                "p (j c) -> p j c", c=128)[:, :, 0:64]
                            with nc.allow_low_precision(reason="f32r 4B"):
                                nc.vector.tensor_copy(dst, src)
                def emit_outproj_qt(qt, scalar_copy=False):
                    ts = slice(qt * 128, (qt + 1) * 128)
                    stage = opool.tile([128, D], BF16, tag="ostage")
                    o_ps = [ap.tile([128, 512], F32, tag="sT", bufs=4,
                                    name=f"o_ps{ch}") for ch in range(2)]
                    for p2 in (1, 0):
                        for ch in range(2):
                            nc.tensor.matmul(
                                o_ps[ch][:], t_pb[p2][:, ts],
                                t_wo[p2][:, ch * 512:(ch + 1) * 512],
                                start=(p2 == 1), stop=(p2 == 0))
                    for ch in range(2):
                        dst = stage[:, ch * 512:(ch + 1) * 512]
                        with nc.allow_low_precision(reason="bf16 out"):
                            if scalar_copy:
                                # tail only: ScalarE is idle after the
                                # last exp; parallels the DVE norm chains
                                nc.scalar.activation(
                                    dst, o_ps[ch][:], AF.Copy)
                            else:
                                nc.vector.tensor_copy(dst, o_ps[ch][:])
                    nc.sync.dma_start(out[ts, :], stage[:])

                nitems = len(items)
                # global PV queue across chunks: chunk q+1's S/exp stream
                # starts while chunk q's PV backlog drains, so ScalarE
                # never pauses at chunk boundaries
                pend = deque()
                drain_mode = [False]
                vleft = [32]

                def drain_policy(q, k):
                    # hysteresis: accumulate PVs, then emit them in dense
                    # runs (ahead of the next S pair) - the back-to-back
                    # matmul bursts keep the PE activity window busy so
                    # the HAM clock gate stays at 8/8; in the last chunk's
                    # second half, drain continuously to flatten the tail
                    if len(pend) >= 14:
                        drain_mode[0] = True
                    if len(pend) <= LOOKAHEAD:
                        drain_mode[0] = False
                    return 4 if drain_mode[0] else 0

                op_queue = []

                def emit_pv(ctx, j, t, pt):
                    if not ctx["accs2"]:
                        ctx["accs2"].extend(
                            ap.tile([128, QC], F32, tag="acc2", bufs=4,
                                    name=f"acc_{jj}")
                            for jj in range(HPC))
                    accs2, seen, qs = ctx["accs2"], ctx["seen"], ctx["qs"]
                    base = (t * HPC + j) * 128
                    seen[j] += 1
                    nc.tensor.matmul(
                        accs2[j][:], t_v1[:, base:base + 128], pt[:],
                        start=(seen[j] == 1), stop=(seen[j] == trips[j]))
                    if seen[j] == trips[j]:
                        # normalize right after the slot's last PV:
                        # denominator is pre-broadcast in rows 64:127
                        p, half = j // 2, j % 2
                        rows = slice(half * 64, (half + 1) * 64)
                        den = nrmpool.tile([64, QC], F32, tag="den")
                        if ctx["q"] == N // QC - 1 and j < 2:
                            # final pair: ScalarE is idle after the last
                            # exp - its copy shortens the DVE-serial
                            # norm chain on the critical tail path
                            nc.scalar.activation(
                                den[:], accs2[j][64:128, :], AF.Copy)
                        else:
                            nc.vector.tensor_copy(
                                den[:], accs2[j][64:128, :])
                        rcp = nrmpool.tile([64, QC], F32, tag="rcp")
                        nc.vector.reciprocal_approx_fast(rcp[:], den[:])
                        with nc.allow_low_precision(reason="f32r 4B"):
                            nc.vector.tensor_mul(
                                t_pb[p][rows, qs], accs2[j][0:64, :],
                                rcp[:])
                        if all(ctx["seen"][jj] == trips[jj]
                               for jj in range(HPC)):
                            # all four slots normalized: the chunk's
                            # output projection is dependency-ready;
                            # spread it one query-tile per item iteration
                            op_queue.extend(
                                range(ctx["q"] * 4, (ctx["q"] + 1) * 4))

                for q in range(N // QC):
                    qs = slice(q * QC, (q + 1) * QC)
                    ctx = {"q": q, "qs": qs, "accs2": [],
                           "seen": [0] * HPC}

                    # chunk 0 carries the 32 V-projection c-steps spread
                    # over its first ~3/4 items (PVs defer until V's PSUM
                    # banks are free)
                    vsched = {}
                    if q == 0:
                        # single c-steps (4 matmuls): paired 8-matmul
                        # V bursts sit ahead of the S matmuls in the PE
                        # FIFO and starve the exp feed (~8.7us of chunk-0
                        # exp gaps measured with pairs)
                        span = max(32, int(nitems * 0.75))
                        for s in range(32):
                            vsched.setdefault(
                                min(1 + s * span // 32, nitems - 1), []
                            ).append(s)

                    k = 0
                    while k < nitems:
                        # S pair + exps FIRST: a drain burst ahead of the
                        # S's in the PE FIFO delays the exp feed by the
                        # burst length (measured as ~12 gaps of 1-3us at
                        # burst-onset/outproj points)
                        batch = items[k:k + 2]
                        sts = []
                        for (j, t) in batch:
                            p, half = j // 2, j % 2
                            rows = slice(half * 64, (half + 1) * 64)
                            sT = ap.tile([128, QC], F32, tag="sT", bufs=4)
                            nc.tensor.matmul(
                                sT[:], t_kT[p][rows, t * 128:(t + 1) * 128],
                                t_qT[p][rows, qs], start=True, stop=True)
                            sts.append(sT)
                        for (j, t), sT in zip(batch, sts):
                            pT = ptpool.tile([128, QC], BF16, tag="pT")
                            nc.scalar.activation(
                                pT[:], sT[:], AF.Exp, scale=0.125,
                                bias=t_vm[:, j * NKT + t: j * NKT + t + 1])
                            pend.append((ctx, j, t, pT))
                        for i in (k, k + 1):
                            for s in vsched.get(i, ()):
                                v_step(s // 8, s % 8)
                                vleft[0] -= 1
                        if vleft[0] == 0:
                            budget = drain_policy(q, k)
                            if op_queue:
                                # an outproj (4 mms) stacks on this
                                # iteration - skip the PV burst so the
                                # S feed isn't starved
                                budget = 0
                            while len(pend) > LOOKAHEAD and budget:
                                emit_pv(*pend.popleft())
                                budget -= 1
                            if op_queue:
                                emit_outproj_qt(op_queue.pop(0))
                        k += 2
                while pend:
                    emit_pv(*pend.popleft())
                # warm-keeper: dense dummy matmuls (dead sT slot, no data
                # deps) execute in parallel with the final norm chains on
                # DVE, so the closing output projection runs at the warm
                # 8/8 clock instead of re-throttled 4/8 (~427ns/mm)
                warm = ap.tile([128, QC], F32, tag="sT", bufs=4)
                for _ in range(12):
                    nc.tensor.matmul(
                        warm[:], t_wo[0][:, 0:128], t_qT[0][:, 0:QC],
                        start=True, stop=True)
                for qt in op_queue:
                    emit_outproj_qt(qt, scalar_copy=True)

    nc.finalize()
    return nc


def _make_plans(trips, vls_by_slot):
    """Greedy pair batching: (t, t+1) share one exp iff every core's vl is
    outside the open interval (128*t, 128*(t+2)) - then one bias column
    describes both tiles on every core."""
    plans = []
    for j in range(HPC):
        plan, t = [], 0
        while t < trips[j]:
            if t + 1 < trips[j] and all(
                    v <= 128 * t or v >= 128 * (t + 2)
                    for v in vls_by_slot[j]):
                plan.append((t, 2))
                t += 2
            else:
                plan.append((t, 1))
                t += 1
        plans.append(plan)
    return plans


def kernel(queries, keys, values, valid_len, Wq, Wk, Wv, Wo):
    global LAST_RESULTS
    queries = np.asarray(queries, dtype=np.float32)
    keys = np.asarray(keys, dtype=np.float32)
    values = np.asarray(values, dtype=np.float32)
    Wq = np.asarray(Wq, dtype=np.float32)
    Wk = np.asarray(Wk, dtype=np.float32)
    Wv = np.asarray(Wv, dtype=np.float32)
    Wo = np.asarray(Wo, dtype=np.float32)
    vl = np.asarray(valid_len).astype(np.int64).reshape(B * H)

    # rank-aligned slot assignment: per batch, heads sorted by vl desc;
    # slot j of the 4 cores of that batch takes ranks 4j..4j+3
    order = {}
    for b in range(B):
        idx = (np.argsort(-vl[b * H:(b + 1) * H], kind="stable") + b * H)
        for cg in range(4):
            order[b * 4 + cg] = [int(idx[4 * j + cg]) for j in range(HPC)]
    trips, vls_by_slot = [], []
    for j in range(HPC):
        vs = [int(vl[order[c][j]]) for c in range(NCORES)]
        vls_by_slot.append(vs)
        m = max(-(-v // 128) for v in vs)
        trips.append(max(1, min(NKT, m)))
    nc = _build_program(tuple(trips))

    in_maps = []
    for c in range(NCORES):
        b = c // 4
        heads = order[c]
        cols = np.concatenate(
            [np.arange((h - b * H) * DH, (h - b * H + 1) * DH) for h in heads])

        def wlayout(w):
            return np.ascontiguousarray(
                w[:, cols].reshape(NDC * 128, 256).astype(NPBF16))

        vm = np.zeros((128, HPC * NKT), np.float32)
        for j, h in enumerate(heads):
            bias = np.where(np.arange(N) < vl[h], 0.0, MASK_BIAS)
            vm[:, j * NKT:(j + 1) * NKT] = bias.reshape(NKT, 128).T

        in_maps.append({
            "xTq": np.ascontiguousarray(queries[b].T.astype(NPBF16)),
            "xTk": np.ascontiguousarray(keys[b].T.astype(NPBF16)),
            "xTv": np.ascontiguousarray(values[b].T.astype(NPBF16)),
            "wq": wlayout(Wq),
            "wk": wlayout(Wk),
            "wv": wlayout(Wv),
            "wo": np.ascontiguousarray(Wo[cols, :]).astype(NPBF16),
            "vmask": vm,
        })

    LAST_RESULTS = run_bass_kernel_spmd(nc, in_maps, list(range(NCORES)))
    res = LAST_RESULTS.results

    out = np.zeros((B, N, D), np.float64)
    for c in range(NCORES):
        out[c // 4] += res[c]["out"].astype(np.float64)
    return out.astype(np.float32)


# revision 57
# speedup vs baseline: 1.0358x; 1.0246x over previous
"""Multi-head attention TRN2 kernel (8 NeuronCores, SPMD).

Problem: B=2, N=2048, D=1024, H=16 heads of dim 64, fp32, per-(b,h)
key-length masking (valid_len, length 32).

Sharding: batch*heads across 8 cores - core c handles batch b=c//4 and 4
heads ("slots", rank-aligned by valid_len so the SPMD trip counts stay
balanced).  Per core:

  phase P: K projection (w chunks contiguous in DRAM so the first matmul
    gates on one 64KB DMA), then Q; first xq chunks prefetched behind the
    xk stream.  Drains split over ScalarE+DVE, attention-chunk-0 first;
    Q's final c-iteration finishes the chunk-0 accumulators first and
    drains them inline, so the first attention S matmul doesn't sit
    behind the other 12 projection matmuls in the PE FIFO (~2us).
  phase A (attention, bf16 operands, f32 PSUM):
    The V projection is STREAMED INTO CHUNK 0 as 32 bank-aligned
    single c-steps, ONE per item over the first 75% - pairing them into
    8-matmul bursts starves the exp feed ~8.7us, and two steps on one
    item measurably costs too (V mms sit ahead of S in the PE FIFO);
    PV-drain bursts are skipped entirely on iterations that carry an
    outproj (same FIFO-starvation mechanism, ~2.5us per boundary).  PSUM: V time-shares the 4 "acc2" banks; the
    attention accumulators allocate lazily after V's last pass, so
    4 sT + 4 V/acc2 = 8 exactly.
    V1 blocks are [V_j (64 cols) | ones (64 cols)] so the PV matmul emits
    the softmax denominator PRE-BROADCAST in PSUM rows 64:127.
    Per 512-query chunk, (slot, key-tile) items go round-robin in
    slot-PAIRS: the two 64-row S matmuls land on disjoint PE row groups
    (tile_position from base_partition) and overlap when slack allows.
    exp(S/8 + bias) on ScalarE - the valid_len mask is a per-partition
    bias column (0/-30000); ScalarE is the pacing engine (~578ns/tile).
    PVs are emitted with hysteresis (accumulate >=14 pending, then drain
    4 ahead of each S pair): the resulting dense back-to-back matmul runs
    keep the PE_HAM activity window busy so the clock gate stays at 8/8 -
    without this the PE sticks at 1.2GHz for 10-80us stretches and the
    kernel is bistable between ~187us and ~220us.  The PV queue is GLOBAL
    across chunks, so chunk q+1's S/exp stream starts while chunk q's
    backlog drains and ScalarE never pauses at chunk boundaries.
    normalize per slot right after its last PV: PSUM->SBUF copy,
    reciprocal_approx_fast (DVE custom op; PSUM input mis-executes, so
    feed it SBUF), one tensor_mul.  When a chunk's last
    norm fires, its output projection becomes dependency-ready and is
    spread one 128-query tile per item iteration (a 16-matmul blob here
    starves ScalarE ~6us per boundary).  Tail: the final pair's den
    copies and all closing stage copies run on ScalarE (idle after the
    last exp), and 12 dummy matmuls overlap the final DVE norm chains so
    the closing output projection runs at the warm 8/8 clock (it
    otherwise re-throttles to ~427ns/mm).
Host sums the 4 per-core partials of each batch element (the unshard for
the row-sharded Wo) and gathers.

Known-bad variants (measured): nt=2 exp batching (2-bank sT slots,
bufs=2), confirmed twice - on the per-chunk AND the global-pend
structure (+30us at nominal clock): the ~76ns/tile exp-overhead saving
is swamped by the coarser 2-slot pipeline; Q chunks 1-3 as in-attention
bursts (sT-slot pinch stalls ScalarE ~3us each, net loss); PSUM matmul
outputs that are not bank-aligned silently corrupt (half-bank V
accumulators); start-of-kernel PE warm-up dummies (they queue behind
const-ap memsets + the ACT table-load DMA, start ~8us, can't complete a
contiguous 3.4us busy window before DMA-ready, and push the first real
matmul 3us later - net loss, the ~1.7us cold-start is floor); capping
PV drains at 2 for each chunk's first 6 iterations (targeting the three
~2.5us boundary gaps) - nominal-clock regression to ~199us: the slower
backlog drain thins the dense PV bursts that keep the clock gate warm.

Identified-but-unbuilt next step: ScalarE exp (~108us) is the floor, and
DVE lacks shift ops for a bit-twiddling exp - BUT the valid-key exp
inputs span only |s/8| <~ 2.5 (sigma ~0.4; Wq/Wk scale 0.02 bounds the
scores), so a two-op custom-DVE polynomial exp (deg-8 Horner + a
select() for the -30000-masked lanes, ~1.2us per [128,512] tile) could
offload ~25% of exp tiles to the half-idle DVE for an estimated
15-20us.  Requires registering new DveOps (OPS + _SUB_OPCODE_FOR_NAME
in concourse.dve_ops, per-NEFF table gen handles the rest) and HW
accuracy validation - a multi-hour project.
"""

import sys
import numpy as np
from collections import deque
from contextlib import ExitStack

sys.path.insert(0, "/opt/trn_rl_repo")

import concourse.bass as bass  # noqa: E402
from concourse import bacc, mybir  # noqa: E402
import concourse.tile as tile  # noqa: E402
from concourse.bass_utils import run_bass_kernel_spmd  # noqa: E402

F32 = mybir.dt.float32
BF16 = mybir.dt.bfloat16
AF = mybir.ActivationFunctionType
NPBF16 = mybir.dt.np(BF16)

B, N, D, H = 2, 2048, 1024, 16
DH = 64
HPC = 4          # heads (slots) per core
NCORES = 8
QC = 512         # q chunk (matmul free dim)
NKT = N // 128   # 16 k tiles
NDC = D // 128   # 8 contraction chunks
MASK_BIAS = -30000.0
LOOKAHEAD = 3    # PV stagger (in items) behind S/exp emission

LAST_RESULTS = None  # BassKernelResults of the most recent run (for tooling)


def _build_program(trips):
    """trips: 4 ints (k-tile count per slot)."""
    nc = bacc.Bacc("TRN2", target_bir_lowering=False, debug=False,
                   num_devices=NCORES)

    xTq = nc.dram_tensor("xTq", [D, N], BF16, kind="ExternalInput")
    xTk = nc.dram_tensor("xTk", [D, N], BF16, kind="ExternalInput")
    xTv = nc.dram_tensor("xTv", [D, N], BF16, kind="ExternalInput")
    wq = nc.dram_tensor("wq", [NDC * 128, 256], BF16, kind="ExternalInput")
    wk = nc.dram_tensor("wk", [NDC * 128, 256], BF16, kind="ExternalInput")
    wv = nc.dram_tensor("wv", [NDC * 128, 256], BF16, kind="ExternalInput")
    wo = nc.dram_tensor("wo", [256, D], BF16, kind="ExternalInput")
    vmask = nc.dram_tensor("vmask", [128, HPC * NKT], F32, kind="ExternalInput")
    out = nc.dram_tensor("out", [N, D], BF16, kind="ExternalOutput")

    # flat item list per chunk: round-robin tiles across slots so adjacent
    # items hit different slots (independent chains)
    items = []
    for r in range(max(trips)):
        for j in range(HPC):
            if r < trips[j]:
                items.append((j, r))

    with tile.TileContext(nc) as tc:
        with ExitStack() as ctx:
            wpool = ctx.enter_context(tc.tile_pool(name="wpool", bufs=1))
            xpool = ctx.enter_context(tc.tile_pool(name="xpool", bufs=6))
            qkpool = ctx.enter_context(tc.tile_pool(name="qkpool", bufs=1))
            v1pool = ctx.enter_context(tc.tile_pool(name="v1pool", bufs=1))
            ptpool = ctx.enter_context(tc.tile_pool(name="ptpool", bufs=40))
            nrmpool = ctx.enter_context(tc.tile_pool(name="nrmpool", bufs=4))
            pbpool = ctx.enter_context(tc.tile_pool(name="pbpool", bufs=1))
            opool = ctx.enter_context(tc.tile_pool(name="opool", bufs=8))

            t_wk = wpool.tile([128, NDC * 256], BF16, tag="wk")
            t_wq = wpool.tile([128, NDC * 256], BF16, tag="wq")
            t_wv = wpool.tile([128, NDC * 256], BF16, tag="wv")
            t_wo = [wpool.tile([128, D], BF16, tag=f"wo{p}", name=f"t_wo{p}")
                    for p in range(2)]
            t_vm = wpool.tile([128, HPC * NKT], F32, tag="vm")

            # K^T/Q^T: [128 dims (2 slots), N] per slot-pair
            t_kT = [qkpool.tile([128, N], BF16, tag=f"kT{p}", name=f"t_kT{p}")
                    for p in range(2)]
            t_qT = [qkpool.tile([128, N], BF16, tag=f"qT{p}", name=f"t_qT{p}")
                    for p in range(2)]
            # V1: per key-tile t, 4 blocks of [V_j (64 cols) | ones (64 cols)]
            t_v1 = v1pool.tile([128, NKT * HPC * 128], BF16, tag="v1")
            # normalized heads^T per slot pair: [128 dims, N]
            t_pb = [pbpool.tile([128, N], BF16, tag=f"pb{p}", name=f"t_pb{p}")
                    for p in range(2)]
            # scratch for the ACT exp-table preload
            t_pre = wpool.tile([1, 1], F32, tag="pre")

            # ---- phase P: projections (K, Q, V) ----
            with tc.tile_pool(name="pp", bufs=8, space="PSUM") as pp:
                xq_pre = []
                for si, (xin, wsb, dsts) in enumerate(
                        ((xTk, t_wk, t_kT), (xTq, t_wq, t_qT))):
                    accs = [pp.tile([128, QC], F32, tag="acc", name=f"acc_{i}")
                            for i in range(8)]
                    for c in range(NDC):
                        if si == 0 and c >= 4:
                            # prefetch the first xq chunks behind the xk
                            # stream so Q matmuls start right after K's
                            xp = xpool.tile([128, N], BF16, tag="xqp",
                                            bufs=4)
                            nc.sync.dma_start(
                                xp[:], xTq[(c - 4) * 128:(c - 3) * 128, :])
                            xq_pre.append(xp)
                        if si == 1 and c < 4:
                            xt = xq_pre[c]
                        else:
                            xt = xpool.tile([128, N], BF16, tag="xt")
                        if si == 0 and c == 0:
                            # split the gating first chunk across 4 DMA
                            # queues so the first matmul fires sooner
                            for sp in range(4):
                                cs = slice(sp * 512, (sp + 1) * 512)
                                nc.sync.dma_start(
                                    xt[:, cs], xin[0:128, cs])
                        elif not (si == 1 and c < 4):
                            nc.sync.dma_start(
                                xt[:], xin[c * 128:(c + 1) * 128, :])
                        if si == 0:
                            # wk sliced per chunk: the first matmul only
                            # gates on 64KB of weights + one x chunk
                            nc.sync.dma_start(
                                t_wk[:, c * 256:(c + 1) * 256],
                                wk[c * 128:(c + 1) * 128, :])
                        if si == 0 and c == 0:
                            # ACT exp-table preload + V1 ones memset, behind
                            # the first input DMAs in trigger order so they
                            # don't delay the x stream
                            nc.scalar.activation(t_pre[:], t_pre[:], AF.Exp)
                            ones_ap = t_v1[:].rearrange(
                                "p (b c) -> p b c", c=128)[:, :, 64:128]
                            nc.vector.memset(ones_ap, 1.0)
                        qq_order = range(4)
                        if si == 1 and c == NDC - 1:
                            # finish the chunk-0 (qq=0) accumulators
                            # first and drain them inline, so the first
                            # attention S matmul doesn't sit behind the
                            # other 12 projection matmuls in the PE FIFO
                            for m in range(2):
                                nc.tensor.matmul(
                                    accs[m * 4][:],
                                    wsb[:, c * 256 + m * 128:
                                        c * 256 + (m + 1) * 128],
                                    xt[:, 0:QC],
                                    start=False, stop=True)
                            with nc.allow_low_precision(reason="f32r 4B"):
                                nc.scalar.activation(
                                    dsts[0][:, 0:QC], accs[0][:], AF.Copy)
                                nc.vector.tensor_copy(
                                    dsts[1][:, 0:QC], accs[4][:])
                            qq_order = range(1, 4)
                        for m in range(2):
                            for qq in qq_order:
                                nc.tensor.matmul(
                                    accs[m * 4 + qq][:],
                                    wsb[:, c * 256 + m * 128:
                                        c * 256 + (m + 1) * 128],
                                    xt[:, qq * QC:(qq + 1) * QC],
                                    start=(c == 0), stop=(c == NDC - 1))
                    wnext, tnext = (wq, t_wq) if si == 0 else (wv, t_wv)
                    for c in range(NDC):
                        nc.sync.dma_start(
                            tnext[:, c * 256:(c + 1) * 256],
                            wnext[c * 128:(c + 1) * 128, :])
                    # drain PSUM->SBUF casts on BOTH ScalarE and DVE, q-chunk
                    # 0 first: the first attention S matmuls gate only on the
                    # chunk-0 casts, so phase A starts ~2.5us earlier
                    order = (0, 4, 1, 5, 2, 6, 3, 7) if si == 0 else (
                        1, 5, 2, 6, 3, 7)
                    for n, i in enumerate(order):
                        dst = dsts[i // 4][:, (i % 4) * QC:(i % 4 + 1) * QC]
                        with nc.allow_low_precision(reason="f32r 4B"):
                            if n % 2 == 0:
                                nc.scalar.activation(dst, accs[i][:], AF.Copy)
                            else:
                                nc.vector.tensor_copy(dst, accs[i][:])
                nc.sync.dma_start(t_wo[0][:], wo[0:128, :])
                nc.sync.dma_start(t_wo[1][:], wo[128:256, :])
                nc.sync.dma_start(t_vm[:], vmask[:])

            # ---- phase A: attention with V projection streamed into chunk
            # 0 (fills PE bubbles while ScalarE chews chunk-0 exps) and
            # fused output projection ----
            with tc.tile_pool(name="ap", bufs=1, space="PSUM") as ap:
                # V projection, split into 16 c-steps interleaved with the
                # chunk-0 S/exp stream.  PSUM: the two 4-bank pass groups
                # come from the same rotation ("acc2" tag) that the
                # attention accumulators use later - acc2 tiles allocate
                # only after V is done, so 4 sT + 4 V banks = 8 exactly.
                vaccs = [None]

                def v_step(sp, c):
                    # sub-pass sp covers 4 key-tiles (bank-aligned [128,512]
                    # slots, V data in cols 0:256), streamed over 8 c-steps
                    g, h = sp // 2, sp % 2
                    if c == 0:
                        vaccs[0] = [ap.tile([128, 512], F32, tag="acc2",
                                            bufs=4, name=f"vacc{sp}_{i}")
                                    for i in range(4)]
                    xt = xpool.tile([128, 512], BF16, tag="xtv")
                    nc.sync.dma_start(
                        xt[:], xTv[c * 128:(c + 1) * 128,
                                   g * 1024 + h * 512:
                                   g * 1024 + (h + 1) * 512])
                    for k in range(4):
                        nc.tensor.matmul(
                            vaccs[0][k][:, 0:256],
                            xt[:, k * 128:(k + 1) * 128],
                            t_wv[:, c * 256:(c + 1) * 256],
                            start=(c == 0), stop=(c == NDC - 1))
                    if c == NDC - 1:
                        for k in range(4):
                            t = g * 8 + h * 4 + k
                            # [128, 4, 64] strided copy: slot j -> V1 block
                            src = vaccs[0][k][:, 0:256].rearrange(
                                "p (j c) -> p j c", c=64)
                            dst = t_v1[:, t * 512:(t + 1) * 512].rearrange(
                                "p (j c) -> p j c", c=128)[:, :, 0:64]
                            with nc.allow_low_precision(reason="f32r 4B"):
                                nc.vector.tensor_copy(dst, src)
                def emit_outproj_qt(qt, scalar_copy=False):
                    ts = slice(qt * 128, (qt + 1) * 128)
                    stage = opool.tile([128, D], BF16, tag="ostage")
                    o_ps = [ap.tile([128, 512], F32, tag="sT", bufs=4,
                                    name=f"o_ps{ch}") for ch in range(2)]
                    for p2 in (1, 0):
                        for ch in range(2):
                            nc.tensor.matmul(
                                o_ps[ch][:], t_pb[p2][:, ts],
                                t_wo[p2][:, ch * 512:(ch + 1) * 512],
                                start=(p2 == 1), stop=(p2 == 0))
                    for ch in range(2):
                        dst = stage[:, ch * 512:(ch + 1) * 512]
                        with nc.allow_low_precision(reason="bf16 out"):
                            if scalar_copy:
                                # tail only: ScalarE is idle after the
                                # last exp; parallels the DVE norm chains
                                nc.scalar.activation(
                                    dst, o_ps[ch][:], AF.Copy)
                            else:
                                nc.vector.tensor_copy(dst, o_ps[ch][:])
                    nc.sync.dma_start(out[ts, :], stage[:])

                nitems = len(items)
                # global PV queue across chunks: chunk q+1's S/exp stream
                # starts while chunk q's PV backlog drains, so ScalarE
                # never pauses at chunk boundaries
                pend = deque()
                drain_mode = [False]
                vleft = [32]

                def drain_policy(q, k):
                    # hysteresis: accumulate PVs, then emit them in dense
                    # runs (ahead of the next S pair) - the back-to-back
                    # matmul bursts keep the PE activity window busy so
                    # the HAM clock gate stays at 8/8; in the last chunk's
                    # second half, drain continuously to flatten the tail
                    if len(pend) >= 14:
                        drain_mode[0] = True
                    if len(pend) <= LOOKAHEAD:
                        drain_mode[0] = False
                    return 4 if drain_mode[0] else 0

                op_queue = []

                def emit_pv(ctx, j, t, pt):
                    if not ctx["accs2"]:
                        ctx["accs2"].extend(
                            ap.tile([128, QC], F32, tag="acc2", bufs=4,
                                    name=f"acc_{jj}")
                            for jj in range(HPC))
                    accs2, seen, qs = ctx["accs2"], ctx["seen"], ctx["qs"]
                    base = (t * HPC + j) * 128
                    seen[j] += 1
                    nc.tensor.matmul(
                        accs2[j][:], t_v1[:, base:base + 128], pt[:],
                        start=(seen[j] == 1), stop=(seen[j] == trips[j]))
                    if seen[j] == trips[j]:
                        # normalize right after the slot's last PV:
                        # denominator is pre-broadcast in rows 64:127
                        p, half = j // 2, j % 2
                        rows = slice(half * 64, (half + 1) * 64)
                        den = nrmpool.tile([64, QC], F32, tag="den")
                        if ctx["q"] == N // QC - 1 and j < 2:
                            # final pair: ScalarE is idle after the last
                            # exp - its copy shortens the DVE-serial
                            # norm chain on the critical tail path
                            nc.scalar.activation(
                                den[:], accs2[j][64:128, :], AF.Copy)
                        else:
                            nc.vector.tensor_copy(
                                den[:], accs2[j][64:128, :])
                        rcp = nrmpool.tile([64, QC], F32, tag="rcp")
                        nc.vector.reciprocal_approx_fast(rcp[:], den[:])
                        with nc.allow_low_precision(reason="f32r 4B"):
                            nc.vector.tensor_mul(
                                t_pb[p][rows, qs], accs2[j][0:64, :],
                                rcp[:])
                        if all(ctx["seen"][jj] == trips[jj]
                               for jj in range(HPC)):
                            # all four slots normalized: the chunk's
                            # output projection is dependency-ready;
                            # spread it one query-tile per item iteration
                            op_queue.extend(
                                range(ctx["q"] * 4, (ctx["q"] + 1) * 4))

                for q in range(N // QC):
                    qs = slice(q * QC, (q + 1) * QC)
                    ctx = {"q": q, "qs": qs, "accs2": [],
                           "seen": [0] * HPC}

                    # chunk 0 carries the 32 V-projection c-steps spread
                    # over its first ~3/4 items (PVs defer until V's PSUM
                    # banks are free)
                    vsched = {}
                    if q == 0:
                        # single c-steps (4 matmuls): paired 8-matmul
                        # V bursts sit ahead of the S matmuls in the PE
                        # FIFO and starve the exp feed (~8.7us of chunk-0
                        # exp gaps measured with pairs)
                        span = max(32, int(nitems * 0.75))
                        for s in range(32):
                            vsched.setdefault(
                                min(1 + s * span // 32, nitems - 1), []
                            ).append(s)

                    k = 0
                    while k < nitems:
                        # S pair + exps FIRST: a drain burst ahead of the
                        # S's in the PE FIFO delays the exp feed by the
                        # burst length (measured as ~12 gaps of 1-3us at
                        # burst-onset/outproj points)
                        batch = items[k:k + 2]
                        sts = []
                        for (j, t) in batch:
                            p, half = j // 2, j % 2
                            rows = slice(half * 64, (half + 1) * 64)
                            sT = ap.tile([128, QC], F32, tag="sT", bufs=4)
                            nc.tensor.matmul(
                                sT[:], t_kT[p][rows, t * 128:(t + 1) * 128],
                                t_qT[p][rows, qs], start=True, stop=True)
                            sts.append(sT)
                        for (j, t), sT in zip(batch, sts):
                            pT = ptpool.tile([128, QC], BF16, tag="pT")
                            nc.scalar.activation(
                                pT[:], sT[:], AF.Exp, scale=0.125,
                                bias=t_vm[:, j * NKT + t: j * NKT + t + 1])
                            pend.append((ctx, j, t, pT))
                        for i in (k, k + 1):
                            for s in vsched.get(i, ()):
                                v_step(s // 8, s % 8)
                                vleft[0] -= 1
                        if vleft[0] == 0:
                            budget = drain_policy(q, k)
                            if op_queue:
                                # an outproj (4 mms) stacks on this
                                # iteration - skip the PV burst so the
                                # S feed isn't starved
                                budget = 0
                            while len(pend) > LOOKAHEAD and budget:
                                emit_pv(*pend.popleft())
                                budget -= 1
                            if op_queue:
                                emit_outproj_qt(op_queue.pop(0))
                        k += 2
                while pend:
                    emit_pv(*pend.popleft())
                # warm-keeper: dense dummy matmuls (dead sT slot, no data
                # deps) execute in parallel with the final norm chains on
                # DVE, so the closing output projection runs at the warm
                # 8/8 clock instead of re-throttled 4/8 (~427ns/mm)
                warm = ap.tile([128, QC], F32, tag="sT", bufs=4)
                for _ in range(12):
                    nc.tensor.matmul(
                        warm[:], t_wo[0][:, 0:128], t_qT[0][:, 0:QC],
                        start=True, stop=True)
                for qt in op_queue:
                    emit_outproj_qt(qt, scalar_copy=True)

    nc.finalize()
    return nc


def _make_plans(trips, vls_by_slot):
    """Greedy pair batching: (t, t+1) share one exp iff every core's vl is
    outside the open interval (128*t, 128*(t+2)) - then one bias column
    describes both tiles on every core."""
    plans = []
    for j in range(HPC):
        plan, t = [], 0
        while t < trips[j]:
            if t + 1 < trips[j] and all(
                    v <= 128 * t or v >= 128 * (t + 2)
                    for v in vls_by_slot[j]):
                plan.append((t, 2))
                t += 2
            else:
                plan.append((t, 1))
                t += 1
        plans.append(plan)
    return plans


def kernel(queries, keys, values, valid_len, Wq, Wk, Wv, Wo):
    global LAST_RESULTS
    queries = np.asarray(queries, dtype=np.float32)
    keys = np.asarray(keys, dtype=np.float32)
    values = np.asarray(values, dtype=np.float32)
    Wq = np.asarray(Wq, dtype=np.float32)
    Wk = np.asarray(Wk, dtype=np.float32)
    Wv = np.asarray(Wv, dtype=np.float32)
    Wo = np.asarray(Wo, dtype=np.float32)
    vl = np.asarray(valid_len).astype(np.int64).reshape(B * H)

    # rank-aligned slot assignment: per batch, heads sorted by vl desc;
    # slot j of the 4 cores of that batch takes ranks 4j..4j+3
    order = {}
    for b in range(B):
        idx = (np.argsort(-vl[b * H:(b + 1) * H], kind="stable") + b * H)
        for cg in range(4):
            order[b * 4 + cg] = [int(idx[4 * j + cg]) for j in range(HPC)]
    trips, vls_by_slot = [], []
    for j in range(HPC):
        vs = [int(vl[order[c][j]]) for c in range(NCORES)]
        vls_by_slot.append(vs)
        m = max(-(-v // 128) for v in vs)
        trips.append(max(1, min(NKT, m)))
    nc = _build_program(tuple(trips))

    in_maps = []
    for c in range(NCORES):
        b = c // 4
        heads = order[c]
        cols = np.concatenate(
            [np.arange((h - b * H) * DH, (h - b * H + 1) * DH) for h in heads])

        def wlayout(w):
            return np.ascontiguousarray(
                w[:, cols].reshape(NDC * 128, 256).astype(NPBF16))

        vm = np.zeros((128, HPC * NKT), np.float32)
        for j, h in enumerate(heads):
            bias = np.where(np.arange(N) < vl[h], 0.0, MASK_BIAS)
            vm[:, j * NKT:(j + 1) * NKT] = bias.reshape(NKT, 128).T

        in_maps.append({
            "xTq": np.ascontiguousarray(queries[b].T.astype(NPBF16)),
            "xTk": np.ascontiguousarray(keys[b].T.astype(NPBF16)),
            "xTv": np.ascontiguousarray(values[b].T.astype(NPBF16)),
            "wq": wlayout(Wq),
            "wk": wlayout(Wk),
            "wv": wlayout(Wv),
            "wo": np.ascontiguousarray(Wo[cols, :]).astype(NPBF16),
            "vmask": vm,
        })

    LAST_RESULTS = run_bass_kernel_spmd(nc, in_maps, list(range(NCORES)))
    res = LAST_RESULTS.results

    out = np.zeros((B, N, D), np.float64)
    for c in range(NCORES):
        out[c // 4] += res[c]["out"].astype(np.float64)
    return out.astype(np.float32)


# revision 58
# speedup vs baseline: 1.0585x; 1.0219x over previous
"""Multi-head attention TRN2 kernel (8 NeuronCores, SPMD).

Problem: B=2, N=2048, D=1024, H=16 heads of dim 64, fp32, per-(b,h)
key-length masking (valid_len, length 32).

Sharding: batch*heads across 8 cores - core c handles batch b=c//4 and 4
heads ("slots", rank-aligned by valid_len so the SPMD trip counts stay
balanced).  Per core:

  phase P: K projection (w chunks contiguous in DRAM so the first matmul
    gates on one 64KB DMA), then Q; first xq chunks prefetched behind the
    xk stream.  Drains split over ScalarE+DVE, attention-chunk-0 first;
    Q's final c-iteration finishes the chunk-0 accumulators first and
    drains them inline, so the first attention S matmul doesn't sit
    behind the other 12 projection matmuls in the PE FIFO (~2us).
  phase A (attention, bf16 operands, f32 PSUM):
    The V projection is STREAMED INTO CHUNK 0 as 32 bank-aligned
    single c-steps, ONE per item over the first 75% - pairing them into
    8-matmul bursts starves the exp feed ~8.7us, and two steps on one
    item measurably costs too (V mms sit ahead of S in the PE FIFO);
    PV-drain bursts are skipped entirely on iterations that carry an
    outproj (same FIFO-starvation mechanism, ~2.5us per boundary).  PSUM: V time-shares the 4 "acc2" banks; the
    attention accumulators allocate lazily after V's last pass, so
    4 sT + 4 V/acc2 = 8 exactly.
    V1 blocks are [V_j (64 cols) | ones (64 cols)] so the PV matmul emits
    the softmax denominator PRE-BROADCAST in PSUM rows 64:127.
    Per 512-query chunk, (slot, key-tile) items go round-robin in
    slot-PAIRS: the two 64-row S matmuls land on disjoint PE row groups
    (tile_position from base_partition) and overlap when slack allows.
    exp(S/8 + bias) on ScalarE - the valid_len mask is a per-partition
    bias column (0/-30000); ScalarE is the pacing engine (~578ns/tile).
    PVs are emitted with hysteresis (accumulate >=14 pending, then drain
    4 ahead of each S pair): the resulting dense back-to-back matmul runs
    keep the PE_HAM activity window busy so the clock gate stays at 8/8 -
    without this the PE sticks at 1.2GHz for 10-80us stretches and the
    kernel is bistable between ~187us and ~220us.  The PV queue is GLOBAL
    across chunks, so chunk q+1's S/exp stream starts while chunk q's
    backlog drains and ScalarE never pauses at chunk boundaries.
    normalize per slot right after its last PV: PSUM->SBUF copy,
    reciprocal_approx_fast (DVE custom op; PSUM input mis-executes, so
    feed it SBUF), one tensor_mul.  When a chunk's last
    norm fires, its output projection becomes dependency-ready and is
    spread one 128-query tile per item iteration (a 16-matmul blob here
    starves ScalarE ~6us per boundary).  Tail: the final pair's den
    copies and all closing stage copies run on ScalarE (idle after the
    last exp), and 12 dummy matmuls overlap the final DVE norm chains so
    the closing output projection runs at the warm 8/8 clock (it
    otherwise re-throttles to ~427ns/mm).
Host sums the 4 per-core partials of each batch element (the unshard for
the row-sharded Wo) and gathers.

Known-bad variants (measured): nt=2 exp batching (2-bank sT slots,
bufs=2), confirmed twice - on the per-chunk AND the global-pend
structure (+30us at nominal clock): the ~76ns/tile exp-overhead saving
is swamped by the coarser 2-slot pipeline; Q chunks 1-3 as in-attention
bursts (sT-slot pinch stalls ScalarE ~3us each, net loss); PSUM matmul
outputs that are not bank-aligned silently corrupt (half-bank V
accumulators); start-of-kernel PE warm-up dummies (they queue behind
const-ap memsets + the ACT table-load DMA, start ~8us, can't complete a
contiguous 3.4us busy window before DMA-ready, and push the first real
matmul 3us later - net loss, the ~1.7us cold-start is floor); capping
PV drains at 2 for each chunk's first 6 iterations (targeting the three
~2.5us boundary gaps) - nominal-clock regression to ~199us: the slower
backlog drain thins the dense PV bursts that keep the clock gate warm.

Identified-but-unbuilt next step: ScalarE exp (~108us) is the floor, and
DVE lacks shift ops for a bit-twiddling exp - BUT the valid-key exp
inputs span only |s/8| <~ 2.5 (sigma ~0.4; Wq/Wk scale 0.02 bounds the
scores), so a two-op custom-DVE polynomial exp (deg-8 Horner + a
select() for the -30000-masked lanes, ~1.2us per [128,512] tile) could
offload ~25% of exp tiles to the half-idle DVE for an estimated
15-20us.  Requires registering new DveOps (OPS + _SUB_OPCODE_FOR_NAME
in concourse.dve_ops, per-NEFF table gen handles the rest) and HW
accuracy validation - a multi-hour project.
"""

import sys
import numpy as np
from collections import deque
from contextlib import ExitStack

sys.path.insert(0, "/opt/trn_rl_repo")

import concourse.bass as bass  # noqa: E402
from concourse import bacc, mybir  # noqa: E402
import concourse.tile as tile  # noqa: E402
from concourse.bass_utils import run_bass_kernel_spmd  # noqa: E402

F32 = mybir.dt.float32
BF16 = mybir.dt.bfloat16
AF = mybir.ActivationFunctionType
NPBF16 = mybir.dt.np(BF16)

B, N, D, H = 2, 2048, 1024, 16
DH = 64
HPC = 4          # heads (slots) per core
NCORES = 8
QC = 512         # q chunk (matmul free dim)
NKT = N // 128   # 16 k tiles
NDC = D // 128   # 8 contraction chunks
MASK_BIAS = -30000.0
LOOKAHEAD = 3    # PV stagger (in items) behind S/exp emission

LAST_RESULTS = None  # BassKernelResults of the most recent run (for tooling)


def _build_program(trips):
    """trips: 4 ints (k-tile count per slot)."""
    nc = bacc.Bacc("TRN2", target_bir_lowering=False, debug=False,
                   num_devices=NCORES)

    xTq = nc.dram_tensor("xTq", [D, N], BF16, kind="ExternalInput")
    xTk = nc.dram_tensor("xTk", [D, N], BF16, kind="ExternalInput")
    xTv = nc.dram_tensor("xTv", [D, N], BF16, kind="ExternalInput")
    wq = nc.dram_tensor("wq", [NDC * 128, 256], BF16, kind="ExternalInput")
    wk = nc.dram_tensor("wk", [NDC * 128, 256], BF16, kind="ExternalInput")
    wv = nc.dram_tensor("wv", [NDC * 128, 256], BF16, kind="ExternalInput")
    wo = nc.dram_tensor("wo", [256, D], BF16, kind="ExternalInput")
    vmask = nc.dram_tensor("vmask", [128, HPC * NKT], F32, kind="ExternalInput")
    out = nc.dram_tensor("out", [N, D], BF16, kind="ExternalOutput")

    # flat item list per chunk: round-robin tiles across slots so adjacent
    # items hit different slots (independent chains)
    items = []
    for r in range(max(trips)):
        for j in range(HPC):
            if r < trips[j]:
                items.append((j, r))

    with tile.TileContext(nc) as tc:
        with ExitStack() as ctx:
            wpool = ctx.enter_context(tc.tile_pool(name="wpool", bufs=1))
            xpool = ctx.enter_context(tc.tile_pool(name="xpool", bufs=6))
            qkpool = ctx.enter_context(tc.tile_pool(name="qkpool", bufs=1))
            v1pool = ctx.enter_context(tc.tile_pool(name="v1pool", bufs=1))
            ptpool = ctx.enter_context(tc.tile_pool(name="ptpool", bufs=40))
            nrmpool = ctx.enter_context(tc.tile_pool(name="nrmpool", bufs=4))
            pbpool = ctx.enter_context(tc.tile_pool(name="pbpool", bufs=1))
            opool = ctx.enter_context(tc.tile_pool(name="opool", bufs=8))

            t_wk = wpool.tile([128, NDC * 256], BF16, tag="wk")
            t_wq = wpool.tile([128, NDC * 256], BF16, tag="wq")
            t_wv = wpool.tile([128, NDC * 256], BF16, tag="wv")
            t_wo = [wpool.tile([128, D], BF16, tag=f"wo{p}", name=f"t_wo{p}")
                    for p in range(2)]
            t_vm = wpool.tile([128, HPC * NKT], F32, tag="vm")

            # K^T/Q^T: [128 dims (2 slots), N] per slot-pair
            t_kT = [qkpool.tile([128, N], BF16, tag=f"kT{p}", name=f"t_kT{p}")
                    for p in range(2)]
            t_qT = [qkpool.tile([128, N], BF16, tag=f"qT{p}", name=f"t_qT{p}")
                    for p in range(2)]
            # V1: per key-tile t, 4 blocks of [V_j (64 cols) | ones (64 cols)]
            t_v1 = v1pool.tile([128, NKT * HPC * 128], BF16, tag="v1")
            # normalized heads^T per slot pair: [128 dims, N]
            t_pb = [pbpool.tile([128, N], BF16, tag=f"pb{p}", name=f"t_pb{p}")
                    for p in range(2)]
            # scratch for the ACT exp-table preload
            t_pre = wpool.tile([1, 1], F32, tag="pre")

            # ---- phase P: projections (K, Q, V) ----
            with tc.tile_pool(name="pp", bufs=8, space="PSUM") as pp:
                xq_pre = []
                for si, (xin, wsb, dsts) in enumerate(
                        ((xTk, t_wk, t_kT), (xTq, t_wq, t_qT))):
                    accs = [pp.tile([128, QC], F32, tag="acc", name=f"acc_{i}")
                            for i in range(8)]
                    for c in range(NDC):
                        if si == 0 and c >= 4:
                            # prefetch the first xq chunks behind the xk
                            # stream so Q matmuls start right after K's
                            xp = xpool.tile([128, N], BF16, tag="xqp",
                                            bufs=4)
                            nc.sync.dma_start(
                                xp[:], xTq[(c - 4) * 128:(c - 3) * 128, :])
                            xq_pre.append(xp)
                        xt0 = None
                        if si == 0 and c == 0:
                            # first chunk as FOUR independent tiles, one
                            # DMA each: the qq-th matmul then waits only
                            # on its own 128KB slice, not the whole 512KB
                            # chunk (Tile tracks deps per tile, so a
                            # single 4-DMA tile gates on all four)
                            xt0 = [xpool.tile([128, QC], BF16, tag="xt0",
                                              bufs=4, name=f"xt0_{sp}")
                                   for sp in range(4)]
                            for sp in range(4):
                                cs = slice(sp * 512, (sp + 1) * 512)
                                nc.sync.dma_start(xt0[sp][:], xin[0:128, cs])
                        elif si == 1 and c < 4:
                            xt = xq_pre[c]
                        else:
                            xt = xpool.tile([128, N], BF16, tag="xt")
                            nc.sync.dma_start(
                                xt[:], xin[c * 128:(c + 1) * 128, :])
                        if si == 0:
                            # wk sliced per chunk: the first matmul only
                            # gates on 64KB of weights + one x chunk
                            nc.sync.dma_start(
                                t_wk[:, c * 256:(c + 1) * 256],
                                wk[c * 128:(c + 1) * 128, :])
                        if si == 0 and c == 0:
                            # ACT exp-table preload + V1 ones memset, behind
                            # the first input DMAs in trigger order so they
                            # don't delay the x stream
                            nc.scalar.activation(t_pre[:], t_pre[:], AF.Exp)
                            ones_ap = t_v1[:].rearrange(
                                "p (b c) -> p b c", c=128)[:, :, 64:128]
                            nc.vector.memset(ones_ap, 1.0)
                        qq_order = range(4)
                        if si == 1 and c == NDC - 1:
                            # finish the chunk-0 (qq=0) accumulators
                            # first and drain them inline, so the first
                            # attention S matmul doesn't sit behind the
                            # other 12 projection matmuls in the PE FIFO
                            for m in range(2):
                                nc.tensor.matmul(
                                    accs[m * 4][:],
                                    wsb[:, c * 256 + m * 128:
                                        c * 256 + (m + 1) * 128],
                                    xt[:, 0:QC],
                                    start=False, stop=True)
                            with nc.allow_low_precision(reason="f32r 4B"):
                                nc.scalar.activation(
                                    dsts[0][:, 0:QC], accs[0][:], AF.Copy)
                                nc.vector.tensor_copy(
                                    dsts[1][:, 0:QC], accs[4][:])
                            qq_order = range(1, 4)
                        for m in range(2):
                            for qq in qq_order:
                                nc.tensor.matmul(
                                    accs[m * 4 + qq][:],
                                    wsb[:, c * 256 + m * 128:
                                        c * 256 + (m + 1) * 128],
                                    xt0[qq][:] if xt0 is not None
                                    else xt[:, qq * QC:(qq + 1) * QC],
                                    start=(c == 0), stop=(c == NDC - 1))
                    wnext, tnext = (wq, t_wq) if si == 0 else (wv, t_wv)
                    for c in range(NDC):
                        nc.sync.dma_start(
                            tnext[:, c * 256:(c + 1) * 256],
                            wnext[c * 128:(c + 1) * 128, :])
                    # drain PSUM->SBUF casts on BOTH ScalarE and DVE, q-chunk
                    # 0 first: the first attention S matmuls gate only on the
                    # chunk-0 casts, so phase A starts ~2.5us earlier
                    order = (0, 4, 1, 5, 2, 6, 3, 7) if si == 0 else (
                        1, 5, 2, 6, 3, 7)
                    for n, i in enumerate(order):
                        dst = dsts[i // 4][:, (i % 4) * QC:(i % 4 + 1) * QC]
                        with nc.allow_low_precision(reason="f32r 4B"):
                            if n % 2 == 0:
                                nc.scalar.activation(dst, accs[i][:], AF.Copy)
                            else:
                                nc.vector.tensor_copy(dst, accs[i][:])
                nc.sync.dma_start(t_wo[0][:], wo[0:128, :])
                nc.sync.dma_start(t_wo[1][:], wo[128:256, :])
                nc.sync.dma_start(t_vm[:], vmask[:])

            # ---- phase A: attention with V projection streamed into chunk
            # 0 (fills PE bubbles while ScalarE chews chunk-0 exps) and
            # fused output projection ----
            with tc.tile_pool(name="ap", bufs=1, space="PSUM") as ap:
                # V projection, split into 16 c-steps interleaved with the
                # chunk-0 S/exp stream.  PSUM: the two 4-bank pass groups
                # come from the same rotation ("acc2" tag) that the
                # attention accumulators use later - acc2 tiles allocate
                # only after V is done, so 4 sT + 4 V banks = 8 exactly.
                vaccs = [None]

                def v_step(sp, c):
                    # sub-pass sp covers 4 key-tiles (bank-aligned [128,512]
                    # slots, V data in cols 0:256), streamed over 8 c-steps
                    g, h = sp // 2, sp % 2
                    if c == 0:
                        vaccs[0] = [ap.tile([128, 512], F32, tag="acc2",
                                            bufs=4, name=f"vacc{sp}_{i}")
                                    for i in range(4)]
                    xt = xpool.tile([128, 512], BF16, tag="xtv")
                    nc.sync.dma_start(
                        xt[:], xTv[c * 128:(c + 1) * 128,
                                   g * 1024 + h * 512:
                                   g * 1024 + (h + 1) * 512])
                    for k in range(4):
                        nc.tensor.matmul(
                            vaccs[0][k][:, 0:256],
                            xt[:, k * 128:(k + 1) * 128],
                            t_wv[:, c * 256:(c + 1) * 256],
                            start=(c == 0), stop=(c == NDC - 1))
                    if c == NDC - 1:
                        for k in range(4):
                            t = g * 8 + h * 4 + k
                            # [128, 4, 64] strided copy: slot j -> V1 block
                            src = vaccs[0][k][:, 0:256].rearrange(
                                "p (j c) -> p j c", c=64)
                            dst = t_v1[:, t * 512:(t + 1) * 512].rearrange(
                                "p (j c) -> p j c", c=128)[:, :, 0:64]
                            with nc.allow_low_precision(reason="f32r 4B"):
                                nc.vector.tensor_copy(dst, src)
                def emit_outproj_qt(qt, scalar_copy=False):
                    ts = slice(qt * 128, (qt + 1) * 128)
                    stage = opool.tile([128, D], BF16, tag="ostage")
                    o_ps = [ap.tile([128, 512], F32, tag="sT", bufs=4,
                                    name=f"o_ps{ch}") for ch in range(2)]
                    for p2 in (1, 0):
                        for ch in range(2):
                            nc.tensor.matmul(
                                o_ps[ch][:], t_pb[p2][:, ts],
                                t_wo[p2][:, ch * 512:(ch + 1) * 512],
                                start=(p2 == 1), stop=(p2 == 0))
                    for ch in range(2):
                        dst = stage[:, ch * 512:(ch + 1) * 512]
                        with nc.allow_low_precision(reason="bf16 out"):
                            if scalar_copy:
                                # tail only: ScalarE is idle after the
                                # last exp; parallels the DVE norm chains
                                nc.scalar.activation(
                                    dst, o_ps[ch][:], AF.Copy)
                            else:
                                nc.vector.tensor_copy(dst, o_ps[ch][:])
                    nc.sync.dma_start(out[ts, :], stage[:])

                nitems = len(items)
                # global PV queue across chunks: chunk q+1's S/exp stream
                # starts while chunk q's PV backlog drains, so ScalarE
                # never pauses at chunk boundaries
                pend = deque()
                drain_mode = [False]
                vleft = [32]

                def drain_policy(q, k):
                    # hysteresis: accumulate PVs, then emit them in dense
                    # runs (ahead of the next S pair) - the back-to-back
                    # matmul bursts keep the PE activity window busy so
                    # the HAM clock gate stays at 8/8; in the last chunk's
                    # second half, drain continuously to flatten the tail
                    if len(pend) >= 14:
                        drain_mode[0] = True
                    if len(pend) <= LOOKAHEAD:
                        drain_mode[0] = False
                    return 4 if drain_mode[0] else 0

                op_queue = []

                def emit_pv(ctx, j, t, pt):
                    if not ctx["accs2"]:
                        ctx["accs2"].extend(
                            ap.tile([128, QC], F32, tag="acc2", bufs=4,
                                    name=f"acc_{jj}")
                            for jj in range(HPC))
                    accs2, seen, qs = ctx["accs2"], ctx["seen"], ctx["qs"]
                    base = (t * HPC + j) * 128
                    seen[j] += 1
                    nc.tensor.matmul(
                        accs2[j][:], t_v1[:, base:base + 128], pt[:],
                        start=(seen[j] == 1), stop=(seen[j] == trips[j]))
                    if seen[j] == trips[j]:
                        # normalize right after the slot's last PV:
                        # denominator is pre-broadcast in rows 64:127
                        p, half = j // 2, j % 2
                        rows = slice(half * 64, (half + 1) * 64)
                        den = nrmpool.tile([64, QC], F32, tag="den")
                        if ctx["q"] == N // QC - 1 and j < 2:
                            # final pair: ScalarE is idle after the last
                            # exp - its copy shortens the DVE-serial
                            # norm chain on the critical tail path
                            nc.scalar.activation(
                                den[:], accs2[j][64:128, :], AF.Copy)
                        else:
                            nc.vector.tensor_copy(
                                den[:], accs2[j][64:128, :])
                        rcp = nrmpool.tile([64, QC], F32, tag="rcp")
                        nc.vector.reciprocal_approx_fast(rcp[:], den[:])
                        with nc.allow_low_precision(reason="f32r 4B"):
                            nc.vector.tensor_mul(
                                t_pb[p][rows, qs], accs2[j][0:64, :],
                                rcp[:])
                        if all(ctx["seen"][jj] == trips[jj]
                               for jj in range(HPC)):
                            # all four slots normalized: the chunk's
                            # output projection is dependency-ready;
                            # spread it one query-tile per item iteration
                            op_queue.extend(
                                range(ctx["q"] * 4, (ctx["q"] + 1) * 4))

                for q in range(N // QC):
                    qs = slice(q * QC, (q + 1) * QC)
                    ctx = {"q": q, "qs": qs, "accs2": [],
                           "seen": [0] * HPC}

                    # chunk 0 carries the 32 V-projection c-steps spread
                    # over its first ~3/4 items (PVs defer until V's PSUM
                    # banks are free)
                    vsched = {}
                    if q == 0:
                        # single c-steps (4 matmuls): paired 8-matmul
                        # V bursts sit ahead of the S matmuls in the PE
                        # FIFO and starve the exp feed (~8.7us of chunk-0
                        # exp gaps measured with pairs)
                        span = max(32, int(nitems * 0.75))
                        for s in range(32):
                            vsched.setdefault(
                                min(1 + s * span // 32, nitems - 1), []
                            ).append(s)

                    k = 0
                    while k < nitems:
                        # S pair + exps FIRST: a drain burst ahead of the
                        # S's in the PE FIFO delays the exp feed by the
                        # burst length (measured as ~12 gaps of 1-3us at
                        # burst-onset/outproj points)
                        batch = items[k:k + 2]
                        sts = []
                        for (j, t) in batch:
                            p, half = j // 2, j % 2
                            rows = slice(half * 64, (half + 1) * 64)
                            sT = ap.tile([128, QC], F32, tag="sT", bufs=4)
                            nc.tensor.matmul(
                                sT[:], t_kT[p][rows, t * 128:(t + 1) * 128],
                                t_qT[p][rows, qs], start=True, stop=True)
                            sts.append(sT)
                        for (j, t), sT in zip(batch, sts):
                            pT = ptpool.tile([128, QC], BF16, tag="pT")
                            nc.scalar.activation(
                                pT[:], sT[:], AF.Exp, scale=0.125,
                                bias=t_vm[:, j * NKT + t: j * NKT + t + 1])
                            pend.append((ctx, j, t, pT))
                        for i in (k, k + 1):
                            for s in vsched.get(i, ()):
                                v_step(s // 8, s % 8)
                                vleft[0] -= 1
                        if vleft[0] == 0:
                            budget = drain_policy(q, k)
                            if op_queue:
                                # an outproj (4 mms) stacks on this
                                # iteration - skip the PV burst so the
                                # S feed isn't starved
                                budget = 0
                            while len(pend) > LOOKAHEAD and budget:
                                emit_pv(*pend.popleft())
                                budget -= 1
                            if op_queue:
                                emit_outproj_qt(op_queue.pop(0))
                        k += 2
                while pend:
                    emit_pv(*pend.popleft())
                # warm-keeper: dense dummy matmuls (dead sT slot, no data
                # deps) execute in parallel with the final norm chains on
                # DVE, so the closing output projection runs at the warm
                # 8/8 clock instead of re-throttled 4/8 (~427ns/mm)
                warm = ap.tile([128, QC], F32, tag="sT", bufs=4)
                for _ in range(12):
                    nc.tensor.matmul(
                        warm[:], t_wo[0][:, 0:128], t_qT[0][:, 0:QC],
                        start=True, stop=True)
                for qt in op_queue:
                    emit_outproj_qt(qt, scalar_copy=True)

    nc.finalize()
    return nc


def _make_plans(trips, vls_by_slot):
    """Greedy pair batching: (t, t+1) share one exp iff every core's vl is
    outside the open interval (128*t, 128*(t+2)) - then one bias column
    describes both tiles on every core."""
    plans = []
    for j in range(HPC):
        plan, t = [], 0
        while t < trips[j]:
            if t + 1 < trips[j] and all(
                    v <= 128 * t or v >= 128 * (t + 2)
                    for v in vls_by_slot[j]):
                plan.append((t, 2))
                t += 2
            else:
                plan.append((t, 1))
                t += 1
        plans.append(plan)
    return plans


def kernel(queries, keys, values, valid_len, Wq, Wk, Wv, Wo):
    global LAST_RESULTS
    queries = np.asarray(queries, dtype=np.float32)
    keys = np.asarray(keys, dtype=np.float32)
    values = np.asarray(values, dtype=np.float32)
    Wq = np.asarray(Wq, dtype=np.float32)
    Wk = np.asarray(Wk, dtype=np.float32)
    Wv = np.asarray(Wv, dtype=np.float32)
    Wo = np.asarray(Wo, dtype=np.float32)
    vl = np.asarray(valid_len).astype(np.int64).reshape(B * H)

    # rank-aligned slot assignment: per batch, heads sorted by vl desc;
    # slot j of the 4 cores of that batch takes ranks 4j..4j+3
    order = {}
    for b in range(B):
        idx = (np.argsort(-vl[b * H:(b + 1) * H], kind="stable") + b * H)
        for cg in range(4):
            order[b * 4 + cg] = [int(idx[4 * j + cg]) for j in range(HPC)]
    trips, vls_by_slot = [], []
    for j in range(HPC):
        vs = [int(vl[order[c][j]]) for c in range(NCORES)]
        vls_by_slot.append(vs)
        m = max(-(-v // 128) for v in vs)
        trips.append(max(1, min(NKT, m)))
    nc = _build_program(tuple(trips))

    in_maps = []
    for c in range(NCORES):
        b = c // 4
        heads = order[c]
        cols = np.concatenate(
            [np.arange((h - b * H) * DH, (h - b * H + 1) * DH) for h in heads])

        def wlayout(w):
            return np.ascontiguousarray(
                w[:, cols].reshape(NDC * 128, 256).astype(NPBF16))

        vm = np.zeros((128, HPC * NKT), np.float32)
        for j, h in enumerate(heads):
            bias = np.where(np.arange(N) < vl[h], 0.0, MASK_BIAS)
            vm[:, j * NKT:(j + 1) * NKT] = bias.reshape(NKT, 128).T

        in_maps.append({
            "xTq": np.ascontiguousarray(queries[b].T.astype(NPBF16)),
            "xTk": np.ascontiguousarray(keys[b].T.astype(NPBF16)),
            "xTv": np.ascontiguousarray(values[b].T.astype(NPBF16)),
            "wq": wlayout(Wq),
            "wk": wlayout(Wk),
            "wv": wlayout(Wv),
            "wo": np.ascontiguousarray(Wo[cols, :]).astype(NPBF16),
            "vmask": vm,
        })

    LAST_RESULTS = run_bass_kernel_spmd(nc, in_maps, list(range(NCORES)))
    res = LAST_RESULTS.results

    out = np.zeros((B, N, D), np.float64)
    for c in range(NCORES):
        out[c // 4] += res[c]["out"].astype(np.float64)
    return out.astype(np.float32)
